# revision 22
# baseline (speedup 1.0000x reference)
"""Trainium2 Bass kernel: GQA multi-head self-attention (B=1, L=4096, D=1024,
16 Q heads, 4 KV heads, head_dim 64, interleaved RoPE, causal softmax).

Sharding: 2 query heads + their (shared) KV head per core, 8 cores.
Each core computes a full-shape partial output Y_c.T = (attn_c @ Wo_c.T).T
(Megatron row-parallel style); the host sums the 8 partials.

Device-side design (per core):
  - x is fed pre-transposed (xT [D, L], fp16) so projection matmuls stream
    natural SBUF tiles; matmul operands are fp16 or fp32r (1 cycle/row on the
    PE, vs 4 for plain fp32), accumulation stays fp32 in PSUM.
  - Q.T/K.T are produced in a "half-split" head-dim order (even dims then odd
    dims per head, via host-permuted weight rows) so RoPE's rotate-pair becomes
    a 32-partition block swap, done with SBUF->SBUF DMAs.
  - Attention runs in the S.T = K @ Q.T orientation: scores land in PSUM as
    [k=128, q=512] tiles (both heads side by side in one 2-bank tile, via
    row-group-packed K=64 matmuls), exp runs on the scalar engine straight out
    of PSUM, and PV uses [V | ones] as the stationary operand so softmax
    denominators come out as row 64 of the PV accumulator for free.
  - No max-subtraction pass: scores are O(1) here, exp cannot overflow, and
    softmax is shift-invariant so the result matches the reference.
  - Emission is software-pipelined: QK^T/exp run two key-blocks ahead of PV,
    and each chunk's normalize + output projection is deferred until the next
    chunk's first two key-blocks are in flight.
"""

import sys

for _p in ("/opt/trn_rl_repo",):
    if _p not in sys.path:
        sys.path.insert(0, _p)

import numpy as np

import concourse.bass as bass
import concourse.bacc as bacc
import concourse.mybir as mybir
import concourse.tile as tile
from concourse.bass_utils import run_bass_kernel_spmd

F32 = mybir.dt.float32
F32R = mybir.dt.float32r
F16 = mybir.dt.float16


def _r(ap):
    return ap.bitcast(F32R)


D_MODEL = 1024
NUM_HEADS = 16
NUM_KV_HEADS = 4
HEAD_DIM = 64
THETA = 10000.0
N_CORES = 8
QC = 512          # query chunk (free dim of S.T tiles per head)
KB = 128          # key block (partition dim of S.T tiles)


def build_kernel(L=4096):
    """One-core SPMD program. Handles its 2 query heads + 1 shared KV head."""
    nc = bacc.Bacc(None, target_bir_lowering=False)
    LC = L // QC          # number of 512-wide l/q chunks
    NT = L // KB          # number of 128-row key blocks / V tiles

    xt = nc.dram_tensor("xt", [D_MODEL, L], F16, kind="ExternalInput")
    wqt = nc.dram_tensor("wqt", [D_MODEL, 128], F16, kind="ExternalInput")
    wkvt = nc.dram_tensor("wkvt", [D_MODEL, 128], F16, kind="ExternalInput")
    wo0 = nc.dram_tensor("wo0", [64, D_MODEL], F16, kind="ExternalInput")
    wo1 = nc.dram_tensor("wo1", [64, D_MODEL], F16, kind="ExternalInput")
    ctab = nc.dram_tensor("ctab", [128, L], F16, kind="ExternalInput")
    s3tab = nc.dram_tensor("s3tab", [128, L], F16, kind="ExternalInput")
    tri = nc.dram_tensor("tri", [128, 128], F16, kind="ExternalInput")
    identlo = nc.dram_tensor("identlo", [128, 64], F16, kind="ExternalInput")
    ones64 = nc.dram_tensor("ones64", [1, 64], F16, kind="ExternalInput")
    yt = nc.dram_tensor("yt", [D_MODEL, L], F16, kind="ExternalOutput")

    with tile.TileContext(nc) as tc:
        with (
            tc.tile_pool(name="consts", bufs=1) as consts,
            tc.tile_pool(name="big", bufs=1) as big,
            tc.tile_pool(name="xin", bufs=12) as xin,
            tc.tile_pool(name="work", bufs=4) as work,
            tc.tile_pool(name="ptp", bufs=6) as ptp,
            tc.tile_pool(name="stp", bufs=2, space="PSUM") as stp,
            tc.tile_pool(name="otp", bufs=2, space="PSUM") as otp,
            tc.tile_pool(name="mp", bufs=2, space="PSUM") as mp,
        ):
            # ---- constants in SBUF ----
            wqt_s = consts.tile([128, 8, 128], F16, tag="wqt")
            wkvt_s = consts.tile([128, 8, 128], F16, tag="wkvt")
            for dc in range(8):
                nc.sync.dma_start(out=wqt_s[:, dc, :], in_=wqt[128 * dc:128 * dc + 128, :])
                nc.sync.dma_start(out=wkvt_s[:, dc, :], in_=wkvt[128 * dc:128 * dc + 128, :])
            wo0_s = consts.tile([64, D_MODEL], F16, tag="wo0")
            wo1_s = consts.tile([64, D_MODEL], F16, tag="wo1")
            ctab_s = consts.tile([128, L], F16, tag="ctab")
            s3tab_s = consts.tile([128, L], F16, tag="s3tab")
            ones64_s = consts.tile([1, 64], F16, tag="ones64")
            tri_s = consts.tile([128, 128], F16, tag="tri")
            identlo_s = consts.tile([128, 64], F16, tag="identlo")

            def load_late_consts():
                nc.sync.dma_start(out=wo0_s, in_=wo0[:, :])
                nc.sync.dma_start(out=wo1_s, in_=wo1[:, :])
                nc.sync.dma_start(out=ones64_s, in_=ones64[:, :])
                nc.sync.dma_start(out=tri_s, in_=tri[:, :])
                nc.sync.dma_start(out=identlo_s, in_=identlo[:, :])

            # ---- persistent per-core activations ----
            qtrope = big.tile([128, L], F16, tag="qtrope")      # [2*64 halfsplit d, L]
            kt2 = big.tile([128, L], F16, tag="kt2")            # K.T duplicated twice
            vn = big.tile([128, NT * 65], F16, tag="vn")        # [V | 1] blocks
            nc.gpsimd.memset(vn, 1.0)

            xtiles = {}

            def proj_dma(lc):
                ls = slice(QC * lc, QC * lc + QC)
                nc.sync.dma_start(out=ctab_s[:, ls], in_=ctab[:, ls])
                nc.sync.dma_start(out=s3tab_s[:, ls], in_=s3tab[:, ls])
                xts = []
                for dc in range(8):
                    xtile = xin.tile([128, QC], F16, tag="xt")
                    nc.sync.dma_start(out=xtile, in_=xt[128 * dc:128 * dc + 128, ls])
                    xts.append(xtile)
                xtiles[lc] = xts

            def proj_compute(lc):
                ls = slice(QC * lc, QC * lc + QC)
                xts = xtiles.pop(lc)
                qt_ps = mp.tile([128, QC], F32, tag="mp")
                kvt_ps = mp.tile([128, QC], F32, tag="mp")
                for dc in range(8):
                    nc.tensor.matmul(qt_ps, wqt_s[:, dc, :], xts[dc],
                                     start=(dc == 0), stop=(dc == 7))
                for dc in range(8):
                    nc.tensor.matmul(kvt_ps, wkvt_s[:, dc, :], xts[dc],
                                     start=(dc == 0), stop=(dc == 7))
                # evacuate PSUM (fp32 -> fp16)
                qtraw = work.tile([128, QC], F16, tag="qtraw")
                kvts = work.tile([128, QC], F16, tag="kvts")
                nc.vector.tensor_copy(qtraw, qt_ps)
                nc.vector.tensor_copy(kvts, kvt_ps)
                # half-split pair swap via SBUF->SBUF DMA (32-row blocks)
                qts = work.tile([128, QC], F16, tag="qts")
                for (a, b) in ((0, 32), (32, 0), (64, 96), (96, 64)):
                    nc.sync.dma_start(out=qts[a:a + 32, :], in_=qtraw[b:b + 32, :])
                kts = work.tile([64, QC], F16, tag="kts")
                nc.sync.dma_start(out=kts[0:32, :], in_=kvts[32:64, :])
                nc.sync.dma_start(out=kts[32:64, :], in_=kvts[0:32, :])
                # RoPE: rot = raw*C + swapped*S3
                t1 = work.tile([128, QC], F16, tag="t1")
                t2 = work.tile([128, QC], F16, tag="t2")
                nc.vector.tensor_mul(t1, qtraw, ctab_s[:, ls])
                nc.vector.tensor_mul(t2, qts, s3tab_s[:, ls])
                nc.vector.tensor_add(qtrope[:, ls], t1, t2)
                t3 = work.tile([64, QC], F16, tag="t1")
                t4 = work.tile([64, QC], F16, tag="t2")
                nc.vector.tensor_mul(t3, kvts[0:64, :], ctab_s[0:64, ls])
                nc.vector.tensor_mul(t4, kts, s3tab_s[0:64, ls])
                nc.vector.tensor_add(kt2[0:64, ls], t3, t4)
                nc.sync.dma_start(out=kt2[64:128, ls], in_=kt2[0:64, ls])
                # V natural layout via PE transpose: kvts[64:128] is V.T [64, 512]
                for t in range(4):
                    vt_ps = mp.tile([128, 64], F16, tag="mp")
                    nc.tensor.transpose(vt_ps, kvts[64:128, 128 * t:128 * t + 128],
                                        identlo_s[64:128, :])
                    blk = 4 * lc + t
                    nc.vector.tensor_copy(vn[:, 65 * blk:65 * blk + 64], vt_ps)

            def make_chunk(qc):
                qs = slice(QC * qc, QC * qc + QC)
                nkb = 4 * (qc + 1)
                state = {}

                def qk(kb):
                    ks = slice(KB * kb, KB * kb + KB)
                    st = stp.tile([128, 2 * QC], F32, tag="st")
                    nc.tensor.matmul(st[:, 0:QC], kt2[0:64, ks], qtrope[0:64, qs],
                                     start=True, stop=True)
                    nc.tensor.matmul(st[:, QC:2 * QC], kt2[64:128, ks], qtrope[64:128, qs],
                                     start=True, stop=True)
                    pt = ptp.tile([128, 2 * QC], F16, tag="pt")
                    nc.scalar.activation(pt, st, mybir.ActivationFunctionType.Exp,
                                         scale=0.125)
                    m = kb - 4 * qc
                    if m >= 0:
                        lo = KB * m
                        nc.vector.tensor_mul(pt[:, lo:lo + KB], pt[:, lo:lo + KB], tri_s)
                        nc.vector.tensor_mul(pt[:, QC + lo:QC + lo + KB],
                                             pt[:, QC + lo:QC + lo + KB], tri_s)
                    return pt

                def pv(kb, pt):
                    if kb == 0:
                        state["ot0"] = otp.tile([65, QC], F32, tag="ot", name="ot0")
                        state["ot1"] = otp.tile([65, QC], F32, tag="ot", name="ot1")
                    m = kb - 4 * qc
                    lo = KB * m if m >= 0 else 0
                    vblk = vn[:, 65 * kb:65 * kb + 65]
                    nc.tensor.matmul(state["ot0"][:, lo:QC], vblk, pt[:, lo:QC],
                                     start=(kb == 0), stop=(kb == nkb - 1),
                                     skip_group_check=True)
                    nc.tensor.matmul(state["ot1"][:, lo:QC], vblk, pt[:, QC + lo:2 * QC],
                                     start=(kb == 0), stop=(kb == nkb - 1),
                                     skip_group_check=True)

                def finish_a():
                    rcs = []
                    for h, ot in enumerate((state["ot0"], state["ot1"])):
                        dst = work.tile([128, QC], F32, tag="dst")
                        nc.vector.tensor_copy(dst[64:65, :], ot[64:65, :])
                        dn = work.tile([128, 4], F32, tag="dn")
                        nc.gpsimd.dma_start(out=dn, in_=dst[64:65, :])
                        rc = work.tile([128, 4], F16, tag="rc")
                        with nc.allow_low_precision(reason="softmax denom recip fp16"):
                            nc.vector.reciprocal(rc, dn)
                        rrow = work.tile([1, QC], F16, tag="rrow")
                        nc.gpsimd.dma_start(out=rrow, in_=rc)
                        rcs.append(rrow)
                    state["rcs"] = rcs

                def finish_b():
                    otns = []
                    for h, ot in enumerate((state["ot0"], state["ot1"])):
                        rbc_ps = mp.tile([64, QC], F32, tag="mp")
                        nc.tensor.matmul(rbc_ps, ones64_s, state["rcs"][h],
                                         start=True, stop=True)
                        rbc = work.tile([64, QC], F32, tag="rbc")
                        nc.vector.tensor_copy(rbc, rbc_ps)
                        otn = work.tile([64, QC], F16, tag=f"otn{h}")
                        nc.vector.tensor_mul(otn, ot[0:64, :], rbc)
                        otns.append(otn)
                    for dc in range(8):
                        yps = mp.tile([128, QC], F32, tag="mp")
                        nc.tensor.matmul(yps, wo0_s[:, 128 * dc:128 * dc + 128], otns[0],
                                         start=True, stop=False)
                        nc.tensor.matmul(yps, wo1_s[:, 128 * dc:128 * dc + 128], otns[1],
                                         start=False, stop=True)
                        ysb = work.tile([128, QC], F16, tag="ysb")
                        nc.vector.tensor_copy(ysb, yps)
                        nc.sync.dma_start(out=yt[128 * dc:128 * dc + 128, qs], in_=ysb)

                return nkb, qk, pv, finish_a, finish_b

            load_late_consts()
            proj_dma(0)
            proj_compute(0)
            if LC > 1:
                proj_dma(1)
                proj_compute(1)
            prev = None
            for qc in range(LC):
                nkb, qk, pv, finish_a, finish_b = make_chunk(qc)
                pts = {}
                pts[0] = qk(0)
                if nkb > 1:
                    pts[1] = qk(1)
                if prev is not None:
                    prev[0]()
                if qc + 2 < LC:
                    proj_dma(qc + 2)
                    proj_compute(qc + 2)
                if prev is not None:
                    prev[1]()
                for kb in range(nkb):
                    if kb + 2 < nkb:
                        pts[kb + 2] = qk(kb + 2)
                    pv(kb, pts.pop(kb))
                prev = (finish_a, finish_b)
            prev[0]()
            prev[1]()

    nc.finalize()
    return nc


def prep_inputs(x, Wq, Wk, Wv, Wo, token_positions, L=4096):
    """Host-side sharding + layout prep. Returns per-core input maps."""
    x = np.asarray(x, dtype=np.float32)
    Wq = np.asarray(Wq, dtype=np.float32)
    Wk = np.asarray(Wk, dtype=np.float32)
    Wv = np.asarray(Wv, dtype=np.float32)
    Wo = np.asarray(Wo, dtype=np.float32)
    pos = np.asarray(token_positions)[0].astype(np.float64)

    xt = np.ascontiguousarray(x[0].T).astype(np.float16)   # [D, L]
    i = np.arange(HEAD_DIM // 2, dtype=np.float64)
    freq = THETA ** (-2.0 * i / HEAD_DIM)                  # [32]
    ang = pos[:, None] * freq[None, :]                     # [L, 32]
    cos = np.cos(ang).T
    sin = np.sin(ang).T
    c64 = np.concatenate([cos, cos], axis=0)               # [64, L]
    s64 = np.concatenate([-sin, sin], axis=0)
    ctab = np.ascontiguousarray(np.concatenate([c64, c64], axis=0)).astype(np.float16)
    s3tab = np.ascontiguousarray(np.concatenate([s64, s64], axis=0)).astype(np.float16)

    perm = np.concatenate([np.arange(0, 64, 2), np.arange(1, 64, 2)])
    tri = (np.arange(128)[None, :] >= np.arange(128)[:, None]).astype(np.float16)
    tri = np.ascontiguousarray(tri)
    ones64 = np.ones((1, 64), dtype=np.float16)
    identlo = np.zeros((128, 64), dtype=np.float16)
    identlo[np.arange(128), np.arange(128) % 64] = 1.0

    in_maps = []
    for c in range(N_CORES):
        h0, h1, g = 2 * c, 2 * c + 1, c // 2
        qrows = np.concatenate([64 * h0 + perm, 64 * h1 + perm])
        wqt = np.ascontiguousarray(Wq[qrows, :].T).astype(np.float16)
        kv = np.concatenate([Wk[64 * g + perm, :], Wv[64 * g:64 * g + 64, :]], axis=0)
        wkvt = np.ascontiguousarray(kv.T).astype(np.float16)
        wo0 = np.ascontiguousarray(Wo[:, 64 * h0:64 * h0 + 64].T).astype(np.float16)
        wo1 = np.ascontiguousarray(Wo[:, 64 * h1:64 * h1 + 64].T).astype(np.float16)
        in_maps.append(dict(xt=xt, wqt=wqt, wkvt=wkvt, wo0=wo0, wo1=wo1,
                            ctab=ctab, s3tab=s3tab, tri=tri,
                            identlo=identlo, ones64=ones64))
    return in_maps


_NC_CACHE = {}


def _get_nc(L=4096):
    if L not in _NC_CACHE:
        _NC_CACHE[L] = build_kernel(L)
    return _NC_CACHE[L]


def kernel(x, Wq, Wk, Wv, Wo, token_positions):
    B, L, D = np.asarray(x).shape
    nc = _get_nc(L)
    in_maps = prep_inputs(x, Wq, Wk, Wv, Wo, token_positions, L=L)
    res = run_bass_kernel_spmd(nc, in_maps, list(range(N_CORES)))
    y = np.zeros((D_MODEL, L), dtype=np.float32)
    for r in res.results:
        y += r["yt"].astype(np.float32)
    return np.ascontiguousarray(y.T)[None].astype(np.float32)


# revision 28
# speedup vs baseline: 1.0401x; 1.0401x over previous
"""Trainium2 Bass kernel: GQA multi-head self-attention (B=1, L=4096, D=1024,
16 Q heads, 4 KV heads, head_dim 64, interleaved RoPE, causal softmax).

Sharding: 2 query heads + their (shared) KV head per core, 8 cores.
Each core computes a full-shape partial output Y_c.T = (attn_c @ Wo_c.T).T
(Megatron row-parallel style); the host sums the 8 partials.

Device-side design (per core):
  - x is fed pre-transposed (xT [D, L], fp16) so projection matmuls stream
    natural SBUF tiles; matmul operands are fp16 or fp32r (1 cycle/row on the
    PE, vs 4 for plain fp32), accumulation stays fp32 in PSUM.
  - Q.T/K.T are produced in a "half-split" head-dim order (even dims then odd
    dims per head, via host-permuted weight rows) so RoPE's rotate-pair becomes
    a 32-partition block swap, done with SBUF->SBUF DMAs.
  - Attention runs in the S.T = K @ Q.T orientation: scores land in PSUM as
    [k=128, q=512] tiles (both heads side by side in one 2-bank tile, via
    row-group-packed K=64 matmuls), exp runs on the scalar engine straight out
    of PSUM, and PV uses [V | ones] as the stationary operand so softmax
    denominators come out as row 64 of the PV accumulator for free.
  - No max-subtraction pass: scores are O(1) here, exp cannot overflow, and
    softmax is shift-invariant so the result matches the reference.
  - Emission is software-pipelined: QK^T/exp run two key-blocks ahead of PV,
    and each chunk's normalize + output projection is deferred until the next
    chunk's first two key-blocks are in flight.
"""

import sys

for _p in ("/opt/trn_rl_repo",):
    if _p not in sys.path:
        sys.path.insert(0, _p)

import numpy as np

import concourse.bacc as bacc
import concourse.mybir as mybir
import concourse.tile as tile
from concourse.bass_utils import run_bass_kernel_spmd

F32 = mybir.dt.float32
F16 = mybir.dt.float16

D_MODEL = 1024
NUM_HEADS = 16
NUM_KV_HEADS = 4
HEAD_DIM = 64
THETA = 10000.0
N_CORES = 8
QC = 512          # query chunk (free dim of S.T tiles per head)
KB = 128          # key block (partition dim of S.T tiles)


def build_kernel(L=4096):
    """One-core SPMD program. Handles its 2 query heads + 1 shared KV head."""
    nc = bacc.Bacc(None, target_bir_lowering=False)
    LC = L // QC          # number of 512-wide l/q chunks
    NT = L // KB          # number of 128-row key blocks / V tiles

    xt = nc.dram_tensor("xt", [D_MODEL, L], F16, kind="ExternalInput")
    wqt = nc.dram_tensor("wqt", [D_MODEL, 128], F16, kind="ExternalInput")
    wkvt = nc.dram_tensor("wkvt", [D_MODEL, 128], F16, kind="ExternalInput")
    wo0 = nc.dram_tensor("wo0", [64, D_MODEL], F16, kind="ExternalInput")
    wo1 = nc.dram_tensor("wo1", [64, D_MODEL], F16, kind="ExternalInput")
    ctab = nc.dram_tensor("ctab", [128, L], F16, kind="ExternalInput")
    s3tab = nc.dram_tensor("s3tab", [128, L], F16, kind="ExternalInput")
    tri = nc.dram_tensor("tri", [128, 128], F16, kind="ExternalInput")
    identlo = nc.dram_tensor("identlo", [128, 64], F16, kind="ExternalInput")
    ones64 = nc.dram_tensor("ones64", [1, 64], F16, kind="ExternalInput")
    yt = nc.dram_tensor("yt", [D_MODEL, L], F16, kind="ExternalOutput")

    with tile.TileContext(nc) as tc:
        with (
            tc.tile_pool(name="consts", bufs=1) as consts,
            tc.tile_pool(name="big", bufs=1) as big,
            tc.tile_pool(name="xin", bufs=24) as xin,
            tc.tile_pool(name="work", bufs=4) as work,
            tc.tile_pool(name="ptp", bufs=12) as ptp,
            tc.tile_pool(name="stp", bufs=2, space="PSUM") as stp,
            tc.tile_pool(name="otp", bufs=2, space="PSUM") as otp,
            tc.tile_pool(name="mp", bufs=2, space="PSUM") as mp,
        ):
            # ---- constants in SBUF ----
            wqt_s = consts.tile([128, 8, 128], F16, tag="wqt")
            wkvt_s = consts.tile([128, 8, 128], F16, tag="wkvt")
            for dc in range(8):
                nc.sync.dma_start(out=wqt_s[:, dc, :], in_=wqt[128 * dc:128 * dc + 128, :])
                nc.sync.dma_start(out=wkvt_s[:, dc, :], in_=wkvt[128 * dc:128 * dc + 128, :])
            wo0_s = consts.tile([64, D_MODEL], F16, tag="wo0")
            wo1_s = consts.tile([64, D_MODEL], F16, tag="wo1")
            ctab_s = consts.tile([128, L], F16, tag="ctab")
            s3tab_s = consts.tile([128, L], F16, tag="s3tab")
            ones64_s = consts.tile([1, 64], F16, tag="ones64")
            tri_s = consts.tile([128, 128], F16, tag="tri")
            identlo_s = consts.tile([128, 64], F16, tag="identlo")

            def load_late_consts():
                nc.sync.dma_start(out=wo0_s, in_=wo0[:, :])
                nc.sync.dma_start(out=wo1_s, in_=wo1[:, :])
                nc.sync.dma_start(out=ones64_s, in_=ones64[:, :])
                nc.sync.dma_start(out=tri_s, in_=tri[:, :])
                nc.sync.dma_start(out=identlo_s, in_=identlo[:, :])

            # ---- persistent per-core activations ----
            qtrope = big.tile([128, L], F16, tag="qtrope")      # [2*64 halfsplit d, L]
            kt2 = big.tile([128, L], F16, tag="kt2")            # K.T duplicated twice
            vn = big.tile([128, NT * 65], F16, tag="vn")        # [V | 1] blocks
            nc.gpsimd.memset(vn, 1.0)

            xtiles = {}

            def proj_dma(lc):
                ls = slice(QC * lc, QC * lc + QC)
                nc.sync.dma_start(out=ctab_s[:, ls], in_=ctab[:, ls])
                nc.sync.dma_start(out=s3tab_s[:, ls], in_=s3tab[:, ls])
                xts = []
                for dc in range(8):
                    xtile = xin.tile([128, QC], F16, tag="xt")
                    nc.sync.dma_start(out=xtile, in_=xt[128 * dc:128 * dc + 128, ls])
                    xts.append(xtile)
                xtiles[lc] = xts

            def proj_compute(lc):
                ls = slice(QC * lc, QC * lc + QC)
                xts = xtiles.pop(lc)
                qt_ps = mp.tile([128, QC], F32, tag="mp")
                kvt_ps = mp.tile([128, QC], F32, tag="mp")
                for dc in range(8):
                    nc.tensor.matmul(qt_ps, wqt_s[:, dc, :], xts[dc],
                                     start=(dc == 0), stop=(dc == 7))
                for dc in range(8):
                    nc.tensor.matmul(kvt_ps, wkvt_s[:, dc, :], xts[dc],
                                     start=(dc == 0), stop=(dc == 7))
                # evacuate PSUM (fp32 -> fp16)
                qtraw = work.tile([128, QC], F16, tag="qtraw")
                kvts = work.tile([128, QC], F16, tag="kvts")
                nc.vector.tensor_copy(qtraw, qt_ps)
                nc.vector.tensor_copy(kvts, kvt_ps)
                # half-split pair swap via SBUF->SBUF DMA (32-row blocks)
                qts = work.tile([128, QC], F16, tag="qts")
                for (a, b) in ((0, 32), (32, 0), (64, 96), (96, 64)):
                    nc.sync.dma_start(out=qts[a:a + 32, :], in_=qtraw[b:b + 32, :])
                kts = work.tile([64, QC], F16, tag="kts")
                nc.sync.dma_start(out=kts[0:32, :], in_=kvts[32:64, :])
                nc.sync.dma_start(out=kts[32:64, :], in_=kvts[0:32, :])
                # RoPE: rot = raw*C + swapped*S3
                t1 = work.tile([128, QC], F16, tag="t1")
                t2 = work.tile([128, QC], F16, tag="t2")
                nc.vector.tensor_mul(t1, qtraw, ctab_s[:, ls])
                nc.vector.tensor_mul(t2, qts, s3tab_s[:, ls])
                nc.vector.tensor_add(qtrope[:, ls], t1, t2)
                t3 = work.tile([64, QC], F16, tag="t1")
                t4 = work.tile([64, QC], F16, tag="t2")
                nc.vector.tensor_mul(t3, kvts[0:64, :], ctab_s[0:64, ls])
                nc.vector.tensor_mul(t4, kts, s3tab_s[0:64, ls])
                nc.vector.tensor_add(kt2[0:64, ls], t3, t4)
                nc.sync.dma_start(out=kt2[64:128, ls], in_=kt2[0:64, ls])
                # V natural layout via PE transpose: kvts[64:128] is V.T [64, 512]
                for t in range(4):
                    vt_ps = mp.tile([128, 64], F16, tag="mp")
                    nc.tensor.transpose(vt_ps, kvts[64:128, 128 * t:128 * t + 128],
                                        identlo_s[64:128, :])
                    blk = 4 * lc + t
                    nc.vector.tensor_copy(vn[:, 65 * blk:65 * blk + 64], vt_ps)

            def make_chunk(qc):
                qs = slice(QC * qc, QC * qc + QC)
                nkb = 4 * (qc + 1)
                state = {}

                def qk(kb):
                    ks = slice(KB * kb, KB * kb + KB)
                    st = stp.tile([128, 2 * QC], F32, tag="st")
                    nc.tensor.matmul(st[:, 0:QC], kt2[0:64, ks], qtrope[0:64, qs],
                                     start=True, stop=True)
                    nc.tensor.matmul(st[:, QC:2 * QC], kt2[64:128, ks], qtrope[64:128, qs],
                                     start=True, stop=True)
                    pt = ptp.tile([128, 2 * QC], F16, tag="pt")
                    nc.scalar.activation(pt, st, mybir.ActivationFunctionType.Exp,
                                         scale=0.125)
                    m = kb - 4 * qc
                    if m >= 0:
                        lo = KB * m
                        nc.vector.tensor_mul(pt[:, lo:lo + KB], pt[:, lo:lo + KB], tri_s)
                        nc.vector.tensor_mul(pt[:, QC + lo:QC + lo + KB],
                                             pt[:, QC + lo:QC + lo + KB], tri_s)
                    return pt

                def pv(kb, pt):
                    if kb == 0:
                        state["ot0"] = otp.tile([65, QC], F32, tag="ot", name="ot0")
                        state["ot1"] = otp.tile([65, QC], F32, tag="ot", name="ot1")
                    m = kb - 4 * qc
                    lo = KB * m if m >= 0 else 0
                    vblk = vn[:, 65 * kb:65 * kb + 65]
                    nc.tensor.matmul(state["ot0"][:, lo:QC], vblk, pt[:, lo:QC],
                                     start=(kb == 0), stop=(kb == nkb - 1),
                                     skip_group_check=True)
                    nc.tensor.matmul(state["ot1"][:, lo:QC], vblk, pt[:, QC + lo:2 * QC],
                                     start=(kb == 0), stop=(kb == nkb - 1),
                                     skip_group_check=True)

                def finish_a():
                    rcs = []
                    for h, ot in enumerate((state["ot0"], state["ot1"])):
                        dst = work.tile([128, QC], F32, tag="dst")
                        nc.vector.tensor_copy(dst[64:65, :], ot[64:65, :])
                        dn = work.tile([128, 4], F32, tag="dn")
                        nc.gpsimd.dma_start(out=dn, in_=dst[64:65, :])
                        rc = work.tile([128, 4], F16, tag="rc")
                        with nc.allow_low_precision(reason="softmax denom recip fp16"):
                            nc.vector.reciprocal(rc, dn)
                        rrow = work.tile([1, QC], F16, tag="rrow")
                        nc.gpsimd.dma_start(out=rrow, in_=rc)
                        rcs.append(rrow)
                    state["rcs"] = rcs

                def finish_b():
                    otns = []
                    for h, ot in enumerate((state["ot0"], state["ot1"])):
                        rbc_ps = mp.tile([64, QC], F32, tag="mp")
                        nc.tensor.matmul(rbc_ps, ones64_s, state["rcs"][h],
                                         start=True, stop=True)
                        rbc = work.tile([64, QC], F32, tag="rbc")
                        nc.vector.tensor_copy(rbc, rbc_ps)
                        otn = work.tile([64, QC], F16, tag=f"otn{h}")
                        nc.vector.tensor_mul(otn, ot[0:64, :], rbc)
                        otns.append(otn)
                    for dc in range(8):
                        yps = mp.tile([128, QC], F32, tag="mp")
                        nc.tensor.matmul(yps, wo0_s[:, 128 * dc:128 * dc + 128], otns[0],
                                         start=True, stop=False)
                        nc.tensor.matmul(yps, wo1_s[:, 128 * dc:128 * dc + 128], otns[1],
                                         start=False, stop=True)
                        ysb = work.tile([128, QC], F16, tag="ysb")
                        nc.vector.tensor_copy(ysb, yps)
                        nc.sync.dma_start(out=yt[128 * dc:128 * dc + 128, qs], in_=ysb)

                return nkb, qk, pv, finish_a, finish_b

            load_late_consts()
            proj_dma(0)
            proj_compute(0)
            if LC > 1:
                proj_dma(1)
                proj_compute(1)
            if LC > 2:
                proj_dma(2)
            prev = None
            for qc in range(LC):
                nkb, qk, pv, finish_a, finish_b = make_chunk(qc)
                pts = {}
                pts[0] = qk(0)
                if nkb > 1:
                    pts[1] = qk(1)
                if prev is not None:
                    prev[0]()
                if qc + 3 < LC:
                    proj_dma(qc + 3)
                if qc + 2 < LC:
                    proj_compute(qc + 2)
                fb_done = prev is None
                for kb in range(nkb):
                    if kb + 2 < nkb:
                        pts[kb + 2] = qk(kb + 2)
                    pv(kb, pts.pop(kb))
                    if kb == 8 and not fb_done:
                        prev[1]()
                        fb_done = True
                if not fb_done:
                    prev[1]()
                prev = (finish_a, finish_b)
            prev[0]()
            prev[1]()

    nc.finalize()
    return nc


def prep_inputs(x, Wq, Wk, Wv, Wo, token_positions, L=4096):
    """Host-side sharding + layout prep. Returns per-core input maps."""
    x = np.asarray(x, dtype=np.float32)
    Wq = np.asarray(Wq, dtype=np.float32)
    Wk = np.asarray(Wk, dtype=np.float32)
    Wv = np.asarray(Wv, dtype=np.float32)
    Wo = np.asarray(Wo, dtype=np.float32)
    pos = np.asarray(token_positions)[0].astype(np.float64)

    xt = np.ascontiguousarray(x[0].T).astype(np.float16)   # [D, L]
    i = np.arange(HEAD_DIM // 2, dtype=np.float64)
    freq = THETA ** (-2.0 * i / HEAD_DIM)                  # [32]
    ang = pos[:, None] * freq[None, :]                     # [L, 32]
    cos = np.cos(ang).T
    sin = np.sin(ang).T
    c64 = np.concatenate([cos, cos], axis=0)               # [64, L]
    s64 = np.concatenate([-sin, sin], axis=0)
    ctab = np.ascontiguousarray(np.concatenate([c64, c64], axis=0)).astype(np.float16)
    s3tab = np.ascontiguousarray(np.concatenate([s64, s64], axis=0)).astype(np.float16)

    perm = np.concatenate([np.arange(0, 64, 2), np.arange(1, 64, 2)])
    tri = (np.arange(128)[None, :] >= np.arange(128)[:, None]).astype(np.float16)
    tri = np.ascontiguousarray(tri)
    ones64 = np.ones((1, 64), dtype=np.float16)
    identlo = np.zeros((128, 64), dtype=np.float16)
    identlo[np.arange(128), np.arange(128) % 64] = 1.0

    in_maps = []
    for c in range(N_CORES):
        h0, h1, g = 2 * c, 2 * c + 1, c // 2
        qrows = np.concatenate([64 * h0 + perm, 64 * h1 + perm])
        wqt = np.ascontiguousarray(Wq[qrows, :].T).astype(np.float16)
        kv = np.concatenate([Wk[64 * g + perm, :], Wv[64 * g:64 * g + 64, :]], axis=0)
        wkvt = np.ascontiguousarray(kv.T).astype(np.float16)
        wo0 = np.ascontiguousarray(Wo[:, 64 * h0:64 * h0 + 64].T).astype(np.float16)
        wo1 = np.ascontiguousarray(Wo[:, 64 * h1:64 * h1 + 64].T).astype(np.float16)
        in_maps.append(dict(xt=xt, wqt=wqt, wkvt=wkvt, wo0=wo0, wo1=wo1,
                            ctab=ctab, s3tab=s3tab, tri=tri,
                            identlo=identlo, ones64=ones64))
    return in_maps


_NC_CACHE = {}


def _get_nc(L=4096):
    if L not in _NC_CACHE:
        _NC_CACHE[L] = build_kernel(L)
    return _NC_CACHE[L]


def kernel(x, Wq, Wk, Wv, Wo, token_positions):
    B, L, D = np.asarray(x).shape
    nc = _get_nc(L)
    in_maps = prep_inputs(x, Wq, Wk, Wv, Wo, token_positions, L=L)
    res = run_bass_kernel_spmd(nc, in_maps, list(range(N_CORES)))
    y = np.zeros((D_MODEL, L), dtype=np.float32)
    for r in res.results:
        y += r["yt"].astype(np.float32)
    return np.ascontiguousarray(y.T)[None].astype(np.float32)


# revision 31
# speedup vs baseline: 1.0466x; 1.0062x over previous
"""Trainium2 Bass kernel: GQA multi-head self-attention (B=1, L=4096, D=1024,
16 Q heads, 4 KV heads, head_dim 64, interleaved RoPE, causal softmax).

Sharding: 2 query heads + their (shared) KV head per core, 8 cores.
Each core computes a full-shape partial output Y_c.T = (attn_c @ Wo_c.T).T
(Megatron row-parallel style); the host sums the 8 partials.

Device-side design (per core):
  - x is fed pre-transposed (xT [D, L], fp16) so projection matmuls stream
    natural SBUF tiles; matmul operands are fp16 or fp32r (1 cycle/row on the
    PE, vs 4 for plain fp32), accumulation stays fp32 in PSUM.
  - Q.T/K.T are produced in a "half-split" head-dim order (even dims then odd
    dims per head, via host-permuted weight rows) so RoPE's rotate-pair becomes
    a 32-partition block swap, done with SBUF->SBUF DMAs.
  - Attention runs in the S.T = K @ Q.T orientation: scores land in PSUM as
    [k=128, q=512] tiles (both heads side by side in one 2-bank tile, via
    row-group-packed K=64 matmuls), exp runs on the scalar engine straight out
    of PSUM, and PV uses [V | ones] as the stationary operand so softmax
    denominators come out as row 64 of the PV accumulator for free.
  - No max-subtraction pass: scores are O(1) here, exp cannot overflow, and
    softmax is shift-invariant so the result matches the reference.
  - Emission is software-pipelined: QK^T/exp run two key-blocks ahead of PV,
    and each chunk's normalize + output projection is deferred until the next
    chunk's first two key-blocks are in flight.
"""

import sys

for _p in ("/opt/trn_rl_repo",):
    if _p not in sys.path:
        sys.path.insert(0, _p)

import numpy as np

import concourse.bacc as bacc
import concourse.mybir as mybir
import concourse.tile as tile
from concourse.bass_utils import run_bass_kernel_spmd

F32 = mybir.dt.float32
F16 = mybir.dt.float16

D_MODEL = 1024
NUM_HEADS = 16
NUM_KV_HEADS = 4
HEAD_DIM = 64
THETA = 10000.0
N_CORES = 8
QC = 512          # query chunk (free dim of S.T tiles per head)
KB = 128          # key block (partition dim of S.T tiles)


def build_kernel(L=4096):
    """One-core SPMD program. Handles its 2 query heads + 1 shared KV head."""
    nc = bacc.Bacc(None, target_bir_lowering=False)
    LC = L // QC          # number of 512-wide l/q chunks
    NT = L // KB          # number of 128-row key blocks / V tiles

    xt = nc.dram_tensor("xt", [D_MODEL, L], F16, kind="ExternalInput")
    wqt = nc.dram_tensor("wqt", [D_MODEL, 128], F16, kind="ExternalInput")
    wkvt = nc.dram_tensor("wkvt", [D_MODEL, 128], F16, kind="ExternalInput")
    wo0 = nc.dram_tensor("wo0", [64, D_MODEL], F16, kind="ExternalInput")
    wo1 = nc.dram_tensor("wo1", [64, D_MODEL], F16, kind="ExternalInput")
    ctab = nc.dram_tensor("ctab", [128, L], F16, kind="ExternalInput")
    s3tab = nc.dram_tensor("s3tab", [128, L], F16, kind="ExternalInput")
    tri = nc.dram_tensor("tri", [128, 128], F16, kind="ExternalInput")
    identlo = nc.dram_tensor("identlo", [128, 64], F16, kind="ExternalInput")
    ones64 = nc.dram_tensor("ones64", [1, 64], F16, kind="ExternalInput")
    yt = nc.dram_tensor("yt", [D_MODEL, L], F16, kind="ExternalOutput")

    with tile.TileContext(nc) as tc:
        with (
            tc.tile_pool(name="consts", bufs=1) as consts,
            tc.tile_pool(name="big", bufs=1) as big,
            tc.tile_pool(name="xin", bufs=24) as xin,
            tc.tile_pool(name="work", bufs=4) as work,
            tc.tile_pool(name="ptp", bufs=12) as ptp,
            tc.tile_pool(name="stp", bufs=2, space="PSUM") as stp,
            tc.tile_pool(name="otp", bufs=2, space="PSUM") as otp,
            tc.tile_pool(name="mp", bufs=2, space="PSUM") as mp,
        ):
            # ---- constants in SBUF ----
            wqt_s = consts.tile([128, 8, 128], F16, tag="wqt")
            wkvt_s = consts.tile([128, 8, 128], F16, tag="wkvt")
            wo0_s = consts.tile([64, D_MODEL], F16, tag="wo0")
            wo1_s = consts.tile([64, D_MODEL], F16, tag="wo1")
            ctab_s = consts.tile([128, L], F16, tag="ctab")
            s3tab_s = consts.tile([128, L], F16, tag="s3tab")
            ones64_s = consts.tile([1, 64], F16, tag="ones64")
            tri_s = consts.tile([128, 128], F16, tag="tri")
            identlo_s = consts.tile([128, 64], F16, tag="identlo")

            def load_late_consts():
                nc.sync.dma_start(out=wo0_s, in_=wo0[:, :])
                nc.sync.dma_start(out=wo1_s, in_=wo1[:, :])
                nc.sync.dma_start(out=ones64_s, in_=ones64[:, :])
                nc.sync.dma_start(out=tri_s, in_=tri[:, :])

            # ---- persistent per-core activations ----
            qtrope = big.tile([128, L], F16, tag="qtrope")      # [2*64 halfsplit d, L]
            kt2 = big.tile([128, L], F16, tag="kt2")            # K.T duplicated twice
            vn = big.tile([128, NT * 65], F16, tag="vn")        # [V | 1] blocks
            nc.gpsimd.memset(vn, 1.0)

            xtiles = {}

            def proj_dma(lc):
                ls = slice(QC * lc, QC * lc + QC)
                xts = []
                for dc in range(8):
                    if lc == 0:
                        nc.sync.dma_start(out=wqt_s[:, dc, :],
                                          in_=wqt[128 * dc:128 * dc + 128, :])
                        nc.sync.dma_start(out=wkvt_s[:, dc, :],
                                          in_=wkvt[128 * dc:128 * dc + 128, :])
                    xtile = xin.tile([128, QC], F16, tag="xt")
                    nc.sync.dma_start(out=xtile, in_=xt[128 * dc:128 * dc + 128, ls])
                    xts.append(xtile)
                nc.sync.dma_start(out=ctab_s[:, ls], in_=ctab[:, ls])
                nc.sync.dma_start(out=s3tab_s[:, ls], in_=s3tab[:, ls])
                xtiles[lc] = xts

            def proj_compute(lc):
                ls = slice(QC * lc, QC * lc + QC)
                xts = xtiles.pop(lc)
                qt_ps = mp.tile([128, QC], F32, tag="mp")
                kvt_ps = mp.tile([128, QC], F32, tag="mp")
                for dc in range(8):
                    nc.tensor.matmul(qt_ps, wqt_s[:, dc, :], xts[dc],
                                     start=(dc == 0), stop=(dc == 7))
                for dc in range(8):
                    nc.tensor.matmul(kvt_ps, wkvt_s[:, dc, :], xts[dc],
                                     start=(dc == 0), stop=(dc == 7))
                # evacuate PSUM (fp32 -> fp16)
                qtraw = work.tile([128, QC], F16, tag="qtraw")
                kvts = work.tile([128, QC], F16, tag="kvts")
                nc.vector.tensor_copy(qtraw, qt_ps)
                nc.vector.tensor_copy(kvts, kvt_ps)
                # half-split pair swap via SBUF->SBUF DMA (32-row blocks)
                qts = work.tile([128, QC], F16, tag="qts")
                for (a, b) in ((0, 32), (32, 0), (64, 96), (96, 64)):
                    nc.sync.dma_start(out=qts[a:a + 32, :], in_=qtraw[b:b + 32, :])
                kts = work.tile([64, QC], F16, tag="kts")
                nc.sync.dma_start(out=kts[0:32, :], in_=kvts[32:64, :])
                nc.sync.dma_start(out=kts[32:64, :], in_=kvts[0:32, :])
                # RoPE: rot = raw*C + swapped*S3
                t1 = work.tile([128, QC], F16, tag="t1")
                t2 = work.tile([128, QC], F16, tag="t2")
                nc.vector.tensor_mul(t1, qtraw, ctab_s[:, ls])
                nc.vector.tensor_mul(t2, qts, s3tab_s[:, ls])
                nc.vector.tensor_add(qtrope[:, ls], t1, t2)
                t3 = work.tile([64, QC], F16, tag="t1")
                t4 = work.tile([64, QC], F16, tag="t2")
                nc.vector.tensor_mul(t3, kvts[0:64, :], ctab_s[0:64, ls])
                nc.vector.tensor_mul(t4, kts, s3tab_s[0:64, ls])
                nc.vector.tensor_add(kt2[0:64, ls], t3, t4)
                nc.sync.dma_start(out=kt2[64:128, ls], in_=kt2[0:64, ls])
                # V natural layout via PE transpose: kvts[64:128] is V.T [64, 512]
                for t in range(4):
                    vt_ps = mp.tile([128, 64], F16, tag="mp")
                    nc.tensor.transpose(vt_ps, kvts[64:128, 128 * t:128 * t + 128],
                                        identlo_s[64:128, :])
                    blk = 4 * lc + t
                    nc.vector.tensor_copy(vn[:, 65 * blk:65 * blk + 64], vt_ps)

            def make_chunk(qc):
                qs = slice(QC * qc, QC * qc + QC)
                nkb = 4 * (qc + 1)
                state = {}

                def qk(kb):
                    ks = slice(KB * kb, KB * kb + KB)
                    st = stp.tile([128, 2 * QC], F32, tag="st")
                    nc.tensor.matmul(st[:, 0:QC], kt2[0:64, ks], qtrope[0:64, qs],
                                     start=True, stop=True)
                    nc.tensor.matmul(st[:, QC:2 * QC], kt2[64:128, ks], qtrope[64:128, qs],
                                     start=True, stop=True)
                    pt = ptp.tile([128, 2 * QC], F16, tag="pt")
                    nc.scalar.activation(pt, st, mybir.ActivationFunctionType.Exp,
                                         scale=0.125)
                    m = kb - 4 * qc
                    if m >= 0:
                        lo = KB * m
                        nc.vector.tensor_mul(pt[:, lo:lo + KB], pt[:, lo:lo + KB], tri_s)
                        nc.vector.tensor_mul(pt[:, QC + lo:QC + lo + KB],
                                             pt[:, QC + lo:QC + lo + KB], tri_s)
                    return pt

                def pv(kb, pt):
                    if kb == 0:
                        state["ot0"] = otp.tile([65, QC], F32, tag="ot", name="ot0")
                        state["ot1"] = otp.tile([65, QC], F32, tag="ot", name="ot1")
                    m = kb - 4 * qc
                    lo = KB * m if m >= 0 else 0
                    vblk = vn[:, 65 * kb:65 * kb + 65]
                    nc.tensor.matmul(state["ot0"][:, lo:QC], vblk, pt[:, lo:QC],
                                     start=(kb == 0), stop=(kb == nkb - 1),
                                     skip_group_check=True)
                    nc.tensor.matmul(state["ot1"][:, lo:QC], vblk, pt[:, QC + lo:2 * QC],
                                     start=(kb == 0), stop=(kb == nkb - 1),
                                     skip_group_check=True)

                def finish_a():
                    rcs = []
                    for h, ot in enumerate((state["ot0"], state["ot1"])):
                        dst = work.tile([128, QC], F32, tag="dst")
                        nc.vector.tensor_copy(dst[64:65, :], ot[64:65, :])
                        dn = work.tile([128, 4], F32, tag="dn")
                        nc.gpsimd.dma_start(out=dn, in_=dst[64:65, :])
                        rc = work.tile([128, 4], F16, tag="rc")
                        with nc.allow_low_precision(reason="softmax denom recip fp16"):
                            nc.vector.reciprocal(rc, dn)
                        rrow = work.tile([1, QC], F16, tag="rrow")
                        nc.gpsimd.dma_start(out=rrow, in_=rc)
                        rcs.append(rrow)
                    state["rcs"] = rcs

                def finish_b():
                    otns = []
                    for h, ot in enumerate((state["ot0"], state["ot1"])):
                        rbc_ps = mp.tile([64, QC], F32, tag="mp")
                        nc.tensor.matmul(rbc_ps, ones64_s, state["rcs"][h],
                                         start=True, stop=True)
                        rbc = work.tile([64, QC], F32, tag="rbc")
                        nc.vector.tensor_copy(rbc, rbc_ps)
                        otn = work.tile([64, QC], F16, tag=f"otn{h}")
                        nc.vector.tensor_mul(otn, ot[0:64, :], rbc)
                        otns.append(otn)
                    for dc in range(8):
                        yps = mp.tile([128, QC], F32, tag="mp")
                        nc.tensor.matmul(yps, wo0_s[:, 128 * dc:128 * dc + 128], otns[0],
                                         start=True, stop=False)
                        nc.tensor.matmul(yps, wo1_s[:, 128 * dc:128 * dc + 128], otns[1],
                                         start=False, stop=True)
                        ysb = work.tile([128, QC], F16, tag="ysb")
                        nc.vector.tensor_copy(ysb, yps)
                        nc.sync.dma_start(out=yt[128 * dc:128 * dc + 128, qs], in_=ysb)

                return nkb, qk, pv, finish_a, finish_b

            nc.sync.dma_start(out=identlo_s, in_=identlo[:, :])
            proj_dma(0)
            proj_compute(0)
            load_late_consts()
            if LC > 1:
                proj_dma(1)
                proj_compute(1)
            if LC > 2:
                proj_dma(2)
            prev = None
            for qc in range(LC):
                nkb, qk, pv, finish_a, finish_b = make_chunk(qc)
                pts = {}
                pts[0] = qk(0)
                if nkb > 1:
                    pts[1] = qk(1)
                if prev is not None:
                    prev[0]()
                if qc + 3 < LC:
                    proj_dma(qc + 3)
                if qc + 2 < LC:
                    proj_compute(qc + 2)
                fb_done = prev is None
                for kb in range(nkb):
                    if kb + 2 < nkb:
                        pts[kb + 2] = qk(kb + 2)
                    pv(kb, pts.pop(kb))
                    if kb == 8 and not fb_done:
                        prev[1]()
                        fb_done = True
                if not fb_done:
                    prev[1]()
                prev = (finish_a, finish_b)
            prev[0]()
            prev[1]()

    nc.finalize()
    return nc


def prep_inputs(x, Wq, Wk, Wv, Wo, token_positions, L=4096):
    """Host-side sharding + layout prep. Returns per-core input maps."""
    x = np.asarray(x, dtype=np.float32)
    Wq = np.asarray(Wq, dtype=np.float32)
    Wk = np.asarray(Wk, dtype=np.float32)
    Wv = np.asarray(Wv, dtype=np.float32)
    Wo = np.asarray(Wo, dtype=np.float32)
    pos = np.asarray(token_positions)[0].astype(np.float64)

    xt = np.ascontiguousarray(x[0].T).astype(np.float16)   # [D, L]
    i = np.arange(HEAD_DIM // 2, dtype=np.float64)
    freq = THETA ** (-2.0 * i / HEAD_DIM)                  # [32]
    ang = pos[:, None] * freq[None, :]                     # [L, 32]
    cos = np.cos(ang).T
    sin = np.sin(ang).T
    c64 = np.concatenate([cos, cos], axis=0)               # [64, L]
    s64 = np.concatenate([-sin, sin], axis=0)
    ctab = np.ascontiguousarray(np.concatenate([c64, c64], axis=0)).astype(np.float16)
    s3tab = np.ascontiguousarray(np.concatenate([s64, s64], axis=0)).astype(np.float16)

    perm = np.concatenate([np.arange(0, 64, 2), np.arange(1, 64, 2)])
    tri = (np.arange(128)[None, :] >= np.arange(128)[:, None]).astype(np.float16)
    tri = np.ascontiguousarray(tri)
    ones64 = np.ones((1, 64), dtype=np.float16)
    identlo = np.zeros((128, 64), dtype=np.float16)
    identlo[np.arange(128), np.arange(128) % 64] = 1.0

    in_maps = []
    for c in range(N_CORES):
        h0, h1, g = 2 * c, 2 * c + 1, c // 2
        qrows = np.concatenate([64 * h0 + perm, 64 * h1 + perm])
        wqt = np.ascontiguousarray(Wq[qrows, :].T).astype(np.float16)
        kv = np.concatenate([Wk[64 * g + perm, :], Wv[64 * g:64 * g + 64, :]], axis=0)
        wkvt = np.ascontiguousarray(kv.T).astype(np.float16)
        wo0 = np.ascontiguousarray(Wo[:, 64 * h0:64 * h0 + 64].T).astype(np.float16)
        wo1 = np.ascontiguousarray(Wo[:, 64 * h1:64 * h1 + 64].T).astype(np.float16)
        in_maps.append(dict(xt=xt, wqt=wqt, wkvt=wkvt, wo0=wo0, wo1=wo1,
                            ctab=ctab, s3tab=s3tab, tri=tri,
                            identlo=identlo, ones64=ones64))
    return in_maps


_NC_CACHE = {}


def _get_nc(L=4096):
    if L not in _NC_CACHE:
        _NC_CACHE[L] = build_kernel(L)
    return _NC_CACHE[L]


def kernel(x, Wq, Wk, Wv, Wo, token_positions):
    B, L, D = np.asarray(x).shape
    nc = _get_nc(L)
    in_maps = prep_inputs(x, Wq, Wk, Wv, Wo, token_positions, L=L)
    res = run_bass_kernel_spmd(nc, in_maps, list(range(N_CORES)))
    y = np.zeros((D_MODEL, L), dtype=np.float32)
    for r in res.results:
        y += r["yt"].astype(np.float32)
    return np.ascontiguousarray(y.T)[None].astype(np.float32)


# revision 32
# speedup vs baseline: 1.0475x; 1.0008x over previous
"""Trainium2 Bass kernel: GQA multi-head self-attention (B=1, L=4096, D=1024,
16 Q heads, 4 KV heads, head_dim 64, interleaved RoPE, causal softmax).

Sharding: 2 query heads + their (shared) KV head per core, 8 cores.
Each core computes a full-shape partial output Y_c.T = (attn_c @ Wo_c.T).T
(Megatron row-parallel style); the host sums the 8 partials.

Device-side design (per core):
  - x is fed pre-transposed (xT [D, L], fp16) so projection matmuls stream
    natural SBUF tiles; matmul operands are fp16 or fp32r (1 cycle/row on the
    PE, vs 4 for plain fp32), accumulation stays fp32 in PSUM.
  - Q.T/K.T are produced in a "half-split" head-dim order (even dims then odd
    dims per head, via host-permuted weight rows) so RoPE's rotate-pair becomes
    a 32-partition block swap, done with SBUF->SBUF DMAs.
  - Attention runs in the S.T = K @ Q.T orientation: scores land in PSUM as
    [k=128, q=512] tiles (both heads side by side in one 2-bank tile, via
    row-group-packed K=64 matmuls), exp runs on the scalar engine straight out
    of PSUM, and PV uses [V | ones] as the stationary operand so softmax
    denominators come out as row 64 of the PV accumulator for free.
  - No max-subtraction pass: scores are O(1) here, exp cannot overflow, and
    softmax is shift-invariant so the result matches the reference.
  - Emission is software-pipelined: QK^T/exp run two key-blocks ahead of PV,
    and each chunk's normalize + output projection is deferred until the next
    chunk's first two key-blocks are in flight.
"""

import sys

for _p in ("/opt/trn_rl_repo",):
    if _p not in sys.path:
        sys.path.insert(0, _p)

import numpy as np

import concourse.bacc as bacc
import concourse.mybir as mybir
import concourse.tile as tile
from concourse.bass_utils import run_bass_kernel_spmd

F32 = mybir.dt.float32
F16 = mybir.dt.float16

D_MODEL = 1024
NUM_HEADS = 16
NUM_KV_HEADS = 4
HEAD_DIM = 64
THETA = 10000.0
N_CORES = 8
QC = 512          # query chunk (free dim of S.T tiles per head)
KB = 128          # key block (partition dim of S.T tiles)


def build_kernel(L=4096):
    """One-core SPMD program. Handles its 2 query heads + 1 shared KV head."""
    nc = bacc.Bacc(None, target_bir_lowering=False)
    LC = L // QC          # number of 512-wide l/q chunks
    NT = L // KB          # number of 128-row key blocks / V tiles

    xt = nc.dram_tensor("xt", [D_MODEL, L], F16, kind="ExternalInput")
    wqt = nc.dram_tensor("wqt", [D_MODEL, 128], F16, kind="ExternalInput")
    wkvt = nc.dram_tensor("wkvt", [D_MODEL, 128], F16, kind="ExternalInput")
    wo0 = nc.dram_tensor("wo0", [64, D_MODEL], F16, kind="ExternalInput")
    wo1 = nc.dram_tensor("wo1", [64, D_MODEL], F16, kind="ExternalInput")
    ctab = nc.dram_tensor("ctab", [128, L], F16, kind="ExternalInput")
    s3tab = nc.dram_tensor("s3tab", [128, L], F16, kind="ExternalInput")
    tri = nc.dram_tensor("tri", [128, 128], F16, kind="ExternalInput")
    identlo = nc.dram_tensor("identlo", [128, 64], F16, kind="ExternalInput")
    ones64 = nc.dram_tensor("ones64", [1, 64], F16, kind="ExternalInput")
    yt = nc.dram_tensor("yt", [D_MODEL, L], F16, kind="ExternalOutput")

    with tile.TileContext(nc) as tc:
        with (
            tc.tile_pool(name="consts", bufs=1) as consts,
            tc.tile_pool(name="big", bufs=1) as big,
            tc.tile_pool(name="xin", bufs=24) as xin,
            tc.tile_pool(name="work", bufs=4) as work,
            tc.tile_pool(name="ptp", bufs=12) as ptp,
            tc.tile_pool(name="stp", bufs=2, space="PSUM") as stp,
            tc.tile_pool(name="otp", bufs=2, space="PSUM") as otp,
            tc.tile_pool(name="mp", bufs=2, space="PSUM") as mp,
        ):
            # ---- constants in SBUF ----
            wqt_s = consts.tile([128, 8, 128], F16, tag="wqt")
            wkvt_s = consts.tile([128, 8, 128], F16, tag="wkvt")
            wo0_s = consts.tile([64, D_MODEL], F16, tag="wo0")
            wo1_s = consts.tile([64, D_MODEL], F16, tag="wo1")
            ctab_s = consts.tile([128, L], F16, tag="ctab")
            s3tab_s = consts.tile([128, L], F16, tag="s3tab")
            ones64_s = consts.tile([1, 64], F16, tag="ones64")
            tri_s = consts.tile([128, 128], F16, tag="tri")
            identlo_s = consts.tile([128, 64], F16, tag="identlo")

            def load_late_consts():
                nc.sync.dma_start(out=wo0_s, in_=wo0[:, :])
                nc.sync.dma_start(out=wo1_s, in_=wo1[:, :])
                nc.sync.dma_start(out=ones64_s, in_=ones64[:, :])
                nc.sync.dma_start(out=tri_s, in_=tri[:, :])

            # ---- persistent per-core activations ----
            qtrope = big.tile([128, L], F16, tag="qtrope")      # [2*64 halfsplit d, L]
            kt2 = big.tile([128, L], F16, tag="kt2")            # K.T duplicated twice
            vn = big.tile([128, NT * 65], F16, tag="vn")        # [V | 1] blocks
            nc.gpsimd.memset(vn, 1.0)

            xtiles = {}

            def proj_dma(lc):
                ls = slice(QC * lc, QC * lc + QC)
                xts = []
                for dc in range(8):
                    if lc == 0:
                        nc.sync.dma_start(out=wqt_s[:, dc, :],
                                          in_=wqt[128 * dc:128 * dc + 128, :])
                        nc.sync.dma_start(out=wkvt_s[:, dc, :],
                                          in_=wkvt[128 * dc:128 * dc + 128, :])
                    xtile = xin.tile([128, QC], F16, tag="xt")
                    nc.sync.dma_start(out=xtile, in_=xt[128 * dc:128 * dc + 128, ls])
                    xts.append(xtile)
                nc.sync.dma_start(out=ctab_s[:, ls], in_=ctab[:, ls])
                nc.sync.dma_start(out=s3tab_s[:, ls], in_=s3tab[:, ls])
                xtiles[lc] = xts

            def proj_compute(lc):
                ls = slice(QC * lc, QC * lc + QC)
                xts = xtiles.pop(lc)
                qt_ps = mp.tile([128, QC], F32, tag="mp")
                kvt_ps = mp.tile([128, QC], F32, tag="mp")
                for dc in range(8):
                    nc.tensor.matmul(qt_ps, wqt_s[:, dc, :], xts[dc],
                                     start=(dc == 0), stop=(dc == 7))
                for dc in range(8):
                    nc.tensor.matmul(kvt_ps, wkvt_s[:, dc, :], xts[dc],
                                     start=(dc == 0), stop=(dc == 7))
                # evacuate PSUM (fp32 -> fp16)
                qtraw = work.tile([128, QC], F16, tag="qtraw")
                kvts = work.tile([128, QC], F16, tag="kvts")
                nc.vector.tensor_copy(qtraw, qt_ps)
                nc.vector.tensor_copy(kvts, kvt_ps)
                # half-split pair swap via SBUF->SBUF DMA (32-row blocks)
                qts = work.tile([128, QC], F16, tag="qts")
                for (a, b) in ((0, 32), (32, 0), (64, 96), (96, 64)):
                    nc.sync.dma_start(out=qts[a:a + 32, :], in_=qtraw[b:b + 32, :])
                kts = work.tile([64, QC], F16, tag="kts")
                nc.sync.dma_start(out=kts[0:32, :], in_=kvts[32:64, :])
                nc.sync.dma_start(out=kts[32:64, :], in_=kvts[0:32, :])
                # RoPE: rot = raw*C + swapped*S3
                t1 = work.tile([128, QC], F16, tag="t1")
                t2 = work.tile([128, QC], F16, tag="t2")
                nc.vector.tensor_mul(t1, qtraw, ctab_s[:, ls])
                nc.vector.tensor_mul(t2, qts, s3tab_s[:, ls])
                nc.vector.tensor_add(qtrope[:, ls], t1, t2)
                t3 = work.tile([64, QC], F16, tag="t1")
                t4 = work.tile([64, QC], F16, tag="t2")
                nc.vector.tensor_mul(t3, kvts[0:64, :], ctab_s[0:64, ls])
                nc.vector.tensor_mul(t4, kts, s3tab_s[0:64, ls])
                nc.vector.tensor_add(kt2[0:64, ls], t3, t4)
                nc.sync.dma_start(out=kt2[64:128, ls], in_=kt2[0:64, ls])
                # V natural layout via PE transpose: kvts[64:128] is V.T [64, 512]
                for t in range(4):
                    vt_ps = mp.tile([128, 64], F16, tag="mp")
                    nc.tensor.transpose(vt_ps, kvts[64:128, 128 * t:128 * t + 128],
                                        identlo_s[64:128, :])
                    blk = 4 * lc + t
                    nc.vector.tensor_copy(vn[:, 65 * blk:65 * blk + 64], vt_ps)

            def make_chunk(qc):
                qs = slice(QC * qc, QC * qc + QC)
                nkb = 4 * (qc + 1)
                state = {}

                def qk(kb):
                    ks = slice(KB * kb, KB * kb + KB)
                    st = stp.tile([128, 2 * QC], F32, tag="st")
                    nc.tensor.matmul(st[:, 0:QC], kt2[0:64, ks], qtrope[0:64, qs],
                                     start=True, stop=True)
                    nc.tensor.matmul(st[:, QC:2 * QC], kt2[64:128, ks], qtrope[64:128, qs],
                                     start=True, stop=True)
                    pt = ptp.tile([128, 2 * QC], F16, tag="pt")
                    nc.scalar.activation(pt, st, mybir.ActivationFunctionType.Exp,
                                         scale=0.125)
                    m = kb - 4 * qc
                    if m >= 0:
                        lo = KB * m
                        nc.vector.tensor_mul(pt[:, lo:lo + KB], pt[:, lo:lo + KB], tri_s)
                        nc.vector.tensor_mul(pt[:, QC + lo:QC + lo + KB],
                                             pt[:, QC + lo:QC + lo + KB], tri_s)
                    return pt

                def pv(kb, pt):
                    if kb == 0:
                        state["ot0"] = otp.tile([65, QC], F32, tag="ot", name="ot0")
                        state["ot1"] = otp.tile([65, QC], F32, tag="ot", name="ot1")
                    m = kb - 4 * qc
                    lo = KB * m if m >= 0 else 0
                    vblk = vn[:, 65 * kb:65 * kb + 65]
                    nc.tensor.matmul(state["ot0"][:, lo:QC], vblk, pt[:, lo:QC],
                                     start=(kb == 0), stop=(kb == nkb - 1),
                                     skip_group_check=True)
                    nc.tensor.matmul(state["ot1"][:, lo:QC], vblk, pt[:, QC + lo:2 * QC],
                                     start=(kb == 0), stop=(kb == nkb - 1),
                                     skip_group_check=True)

                def finish_a(eng=None):
                    eng = eng or nc.gpsimd
                    rcs = []
                    for h, ot in enumerate((state["ot0"], state["ot1"])):
                        dst = work.tile([128, QC], F32, tag="dst")
                        nc.vector.tensor_copy(dst[64:65, :], ot[64:65, :])
                        dn = work.tile([128, 4], F32, tag="dn")
                        eng.dma_start(out=dn, in_=dst[64:65, :])
                        rc = work.tile([128, 4], F16, tag="rc")
                        with nc.allow_low_precision(reason="softmax denom recip fp16"):
                            nc.vector.reciprocal(rc, dn)
                        rrow = work.tile([1, QC], F16, tag="rrow")
                        eng.dma_start(out=rrow, in_=rc)
                        rcs.append(rrow)
                    state["rcs"] = rcs

                def finish_b():
                    otns = []
                    for h, ot in enumerate((state["ot0"], state["ot1"])):
                        rbc_ps = mp.tile([64, QC], F32, tag="mp")
                        nc.tensor.matmul(rbc_ps, ones64_s, state["rcs"][h],
                                         start=True, stop=True)
                        rbc = work.tile([64, QC], F32, tag="rbc")
                        nc.vector.tensor_copy(rbc, rbc_ps)
                        otn = work.tile([64, QC], F16, tag=f"otn{h}")
                        nc.vector.tensor_mul(otn, ot[0:64, :], rbc)
                        otns.append(otn)
                    for dc in range(8):
                        yps = mp.tile([128, QC], F32, tag="mp")
                        nc.tensor.matmul(yps, wo0_s[:, 128 * dc:128 * dc + 128], otns[0],
                                         start=True, stop=False)
                        nc.tensor.matmul(yps, wo1_s[:, 128 * dc:128 * dc + 128], otns[1],
                                         start=False, stop=True)
                        ysb = work.tile([128, QC], F16, tag="ysb")
                        nc.vector.tensor_copy(ysb, yps)
                        nc.sync.dma_start(out=yt[128 * dc:128 * dc + 128, qs], in_=ysb)

                return nkb, qk, pv, finish_a, finish_b

            nc.sync.dma_start(out=identlo_s, in_=identlo[:, :])
            proj_dma(0)
            proj_compute(0)
            load_late_consts()
            if LC > 1:
                proj_dma(1)
                proj_compute(1)
            if LC > 2:
                proj_dma(2)
            prev = None
            for qc in range(LC):
                nkb, qk, pv, finish_a, finish_b = make_chunk(qc)
                pts = {}
                pts[0] = qk(0)
                if nkb > 1:
                    pts[1] = qk(1)
                if prev is not None:
                    prev[0]()
                if qc + 3 < LC:
                    proj_dma(qc + 3)
                if qc + 2 < LC:
                    proj_compute(qc + 2)
                fb_done = prev is None
                fb_kb = min(8, nkb - 2)
                for kb in range(nkb):
                    if kb + 2 < nkb:
                        pts[kb + 2] = qk(kb + 2)
                    pv(kb, pts.pop(kb))
                    if kb == fb_kb and not fb_done:
                        prev[1]()
                        fb_done = True
                if not fb_done:
                    prev[1]()
                prev = (finish_a, finish_b)
            prev[0](nc.sync)
            prev[1]()

    nc.finalize()
    return nc


def prep_inputs(x, Wq, Wk, Wv, Wo, token_positions, L=4096):
    """Host-side sharding + layout prep. Returns per-core input maps."""
    x = np.asarray(x, dtype=np.float32)
    Wq = np.asarray(Wq, dtype=np.float32)
    Wk = np.asarray(Wk, dtype=np.float32)
    Wv = np.asarray(Wv, dtype=np.float32)
    Wo = np.asarray(Wo, dtype=np.float32)
    pos = np.asarray(token_positions)[0].astype(np.float64)

    xt = np.ascontiguousarray(x[0].T).astype(np.float16)   # [D, L]
    i = np.arange(HEAD_DIM // 2, dtype=np.float64)
    freq = THETA ** (-2.0 * i / HEAD_DIM)                  # [32]
    ang = pos[:, None] * freq[None, :]                     # [L, 32]
    cos = np.cos(ang).T
    sin = np.sin(ang).T
    c64 = np.concatenate([cos, cos], axis=0)               # [64, L]
    s64 = np.concatenate([-sin, sin], axis=0)
    ctab = np.ascontiguousarray(np.concatenate([c64, c64], axis=0)).astype(np.float16)
    s3tab = np.ascontiguousarray(np.concatenate([s64, s64], axis=0)).astype(np.float16)

    perm = np.concatenate([np.arange(0, 64, 2), np.arange(1, 64, 2)])
    tri = (np.arange(128)[None, :] >= np.arange(128)[:, None]).astype(np.float16)
    tri = np.ascontiguousarray(tri)
    ones64 = np.ones((1, 64), dtype=np.float16)
    identlo = np.zeros((128, 64), dtype=np.float16)
    identlo[np.arange(128), np.arange(128) % 64] = 1.0

    in_maps = []
    for c in range(N_CORES):
        h0, h1, g = 2 * c, 2 * c + 1, c // 2
        qrows = np.concatenate([64 * h0 + perm, 64 * h1 + perm])
        wqt = np.ascontiguousarray(Wq[qrows, :].T).astype(np.float16)
        kv = np.concatenate([Wk[64 * g + perm, :], Wv[64 * g:64 * g + 64, :]], axis=0)
        wkvt = np.ascontiguousarray(kv.T).astype(np.float16)
        wo0 = np.ascontiguousarray(Wo[:, 64 * h0:64 * h0 + 64].T).astype(np.float16)
        wo1 = np.ascontiguousarray(Wo[:, 64 * h1:64 * h1 + 64].T).astype(np.float16)
        in_maps.append(dict(xt=xt, wqt=wqt, wkvt=wkvt, wo0=wo0, wo1=wo1,
                            ctab=ctab, s3tab=s3tab, tri=tri,
                            identlo=identlo, ones64=ones64))
    return in_maps


_NC_CACHE = {}


def _get_nc(L=4096):
    if L not in _NC_CACHE:
        _NC_CACHE[L] = build_kernel(L)
    return _NC_CACHE[L]


def kernel(x, Wq, Wk, Wv, Wo, token_positions):
    B, L, D = np.asarray(x).shape
    nc = _get_nc(L)
    in_maps = prep_inputs(x, Wq, Wk, Wv, Wo, token_positions, L=L)
    res = run_bass_kernel_spmd(nc, in_maps, list(range(N_CORES)))
    y = np.zeros((D_MODEL, L), dtype=np.float32)
    for r in res.results:
        y += r["yt"].astype(np.float32)
    return np.ascontiguousarray(y.T)[None].astype(np.float32)


# revision 34
# speedup vs baseline: 1.0480x; 1.0005x over previous
"""Trainium2 Bass kernel: GQA multi-head self-attention (B=1, L=4096, D=1024,
16 Q heads, 4 KV heads, head_dim 64, interleaved RoPE, causal softmax).

Sharding: 2 query heads + their (shared) KV head per core, 8 cores.
Each core computes a full-shape partial output Y_c.T = (attn_c @ Wo_c.T).T
(Megatron row-parallel style); the host sums the 8 partials.

Device-side design (per core):
  - x is fed pre-transposed (xT [D, L], fp16) so projection matmuls stream
    natural SBUF tiles; matmul operands are fp16 or fp32r (1 cycle/row on the
    PE, vs 4 for plain fp32), accumulation stays fp32 in PSUM.
  - Q.T/K.T are produced in a "half-split" head-dim order (even dims then odd
    dims per head, via host-permuted weight rows) so RoPE's rotate-pair becomes
    a 32-partition block swap, done with SBUF->SBUF DMAs.
  - Attention runs in the S.T = K @ Q.T orientation: scores land in PSUM as
    [k=128, q=512] tiles (both heads side by side in one 2-bank tile, via
    row-group-packed K=64 matmuls), exp runs on the scalar engine straight out
    of PSUM, and PV uses [V | ones] as the stationary operand so softmax
    denominators come out as row 64 of the PV accumulator for free.
  - No max-subtraction pass: scores are O(1) here, exp cannot overflow, and
    softmax is shift-invariant so the result matches the reference.
  - Emission is software-pipelined: QK^T/exp run two key-blocks ahead of PV,
    and each chunk's normalize + output projection is deferred until the next
    chunk's first two key-blocks are in flight.
"""

import sys

for _p in ("/opt/trn_rl_repo",):
    if _p not in sys.path:
        sys.path.insert(0, _p)

import numpy as np

import concourse.bacc as bacc
import concourse.mybir as mybir
import concourse.tile as tile
from concourse.bass_utils import run_bass_kernel_spmd

F32 = mybir.dt.float32
F16 = mybir.dt.float16

D_MODEL = 1024
NUM_HEADS = 16
NUM_KV_HEADS = 4
HEAD_DIM = 64
THETA = 10000.0
N_CORES = 8
QC = 512          # query chunk (free dim of S.T tiles per head)
KB = 128          # key block (partition dim of S.T tiles)


def build_kernel(L=4096):
    """One-core SPMD program. Handles its 2 query heads + 1 shared KV head."""
    nc = bacc.Bacc(None, target_bir_lowering=False)
    LC = L // QC          # number of 512-wide l/q chunks
    NT = L // KB          # number of 128-row key blocks / V tiles

    xt = nc.dram_tensor("xt", [D_MODEL, L], F16, kind="ExternalInput")
    wqt = nc.dram_tensor("wqt", [D_MODEL, 128], F16, kind="ExternalInput")
    wkvt = nc.dram_tensor("wkvt", [D_MODEL, 128], F16, kind="ExternalInput")
    wo0 = nc.dram_tensor("wo0", [64, D_MODEL], F16, kind="ExternalInput")
    wo1 = nc.dram_tensor("wo1", [64, D_MODEL], F16, kind="ExternalInput")
    ctab = nc.dram_tensor("ctab", [128, L], F16, kind="ExternalInput")
    s3tab = nc.dram_tensor("s3tab", [128, L], F16, kind="ExternalInput")
    tri = nc.dram_tensor("tri", [128, 128], F16, kind="ExternalInput")
    identlo = nc.dram_tensor("identlo", [128, 64], F16, kind="ExternalInput")
    ones64 = nc.dram_tensor("ones64", [1, 64], F16, kind="ExternalInput")
    yt = nc.dram_tensor("yt", [D_MODEL, L], F16, kind="ExternalOutput")

    with tile.TileContext(nc) as tc:
        with (
            tc.tile_pool(name="consts", bufs=1) as consts,
            tc.tile_pool(name="big", bufs=1) as big,
            tc.tile_pool(name="xin", bufs=24) as xin,
            tc.tile_pool(name="work", bufs=4) as work,
            tc.tile_pool(name="ptp", bufs=12) as ptp,
            tc.tile_pool(name="stp", bufs=2, space="PSUM") as stp,
            tc.tile_pool(name="otp", bufs=2, space="PSUM") as otp,
            tc.tile_pool(name="mp", bufs=2, space="PSUM") as mp,
        ):
            # ---- constants in SBUF ----
            wqt_s = consts.tile([128, 8, 128], F16, tag="wqt")
            wkvt_s = consts.tile([128, 8, 128], F16, tag="wkvt")
            wo0_s = consts.tile([64, D_MODEL], F16, tag="wo0")
            wo1_s = consts.tile([64, D_MODEL], F16, tag="wo1")
            ctab_s = consts.tile([128, L], F16, tag="ctab")
            s3tab_s = consts.tile([128, L], F16, tag="s3tab")
            ones64_s = consts.tile([1, 64], F16, tag="ones64")
            tri_s = consts.tile([128, 128], F16, tag="tri")
            identlo_s = consts.tile([128, 64], F16, tag="identlo")

            def load_late_consts():
                nc.sync.dma_start(out=wo0_s, in_=wo0[:, :])
                nc.sync.dma_start(out=wo1_s, in_=wo1[:, :])
                nc.sync.dma_start(out=ones64_s, in_=ones64[:, :])
                nc.sync.dma_start(out=tri_s, in_=tri[:, :])

            # ---- persistent per-core activations ----
            qtrope = big.tile([128, L], F16, tag="qtrope")      # [2*64 halfsplit d, L]
            kt2 = big.tile([128, L], F16, tag="kt2")            # K.T duplicated twice
            vn = big.tile([128, NT * 65], F16, tag="vn")        # [V | 1] blocks
            nc.gpsimd.memset(vn, 1.0)

            xtiles = {}

            def proj_dma(lc):
                ls = slice(QC * lc, QC * lc + QC)
                xts = []
                for dc in range(8):
                    if lc == 0:
                        nc.sync.dma_start(out=wqt_s[:, dc, :],
                                          in_=wqt[128 * dc:128 * dc + 128, :])
                        nc.sync.dma_start(out=wkvt_s[:, dc, :],
                                          in_=wkvt[128 * dc:128 * dc + 128, :])
                    xtile = xin.tile([128, QC], F16, tag="xt")
                    nc.sync.dma_start(out=xtile, in_=xt[128 * dc:128 * dc + 128, ls])
                    xts.append(xtile)
                nc.sync.dma_start(out=ctab_s[:, ls], in_=ctab[:, ls])
                nc.sync.dma_start(out=s3tab_s[:, ls], in_=s3tab[:, ls])
                xtiles[lc] = xts

            def proj_compute(lc):
                ls = slice(QC * lc, QC * lc + QC)
                xts = xtiles.pop(lc)
                qt_ps = mp.tile([128, QC], F32, tag="mp")
                kvt_ps = mp.tile([128, QC], F32, tag="mp")
                for dc in range(8):
                    nc.tensor.matmul(qt_ps, wqt_s[:, dc, :], xts[dc],
                                     start=(dc == 0), stop=(dc == 7))
                for dc in range(8):
                    nc.tensor.matmul(kvt_ps, wkvt_s[:, dc, :], xts[dc],
                                     start=(dc == 0), stop=(dc == 7))
                # evacuate PSUM (fp32 -> fp16)
                qtraw = work.tile([128, QC], F16, tag="qtraw")
                kvts = work.tile([128, QC], F16, tag="kvts")
                nc.vector.tensor_copy(qtraw, qt_ps)
                nc.vector.tensor_copy(kvts, kvt_ps)
                # half-split pair swap via SBUF->SBUF DMA (32-row blocks)
                qts = work.tile([128, QC], F16, tag="qts")
                for (a, b) in ((0, 32), (32, 0), (64, 96), (96, 64)):
                    nc.sync.dma_start(out=qts[a:a + 32, :], in_=qtraw[b:b + 32, :])
                kts = work.tile([64, QC], F16, tag="kts")
                nc.sync.dma_start(out=kts[0:32, :], in_=kvts[32:64, :])
                nc.sync.dma_start(out=kts[32:64, :], in_=kvts[0:32, :])
                # RoPE: rot = raw*C + swapped*S3
                t1 = work.tile([128, QC], F16, tag="t1")
                t2 = work.tile([128, QC], F16, tag="t2")
                nc.vector.tensor_mul(t1, qtraw, ctab_s[:, ls])
                nc.vector.tensor_mul(t2, qts, s3tab_s[:, ls])
                nc.vector.tensor_add(qtrope[:, ls], t1, t2)
                t3 = work.tile([64, QC], F16, tag="t1")
                t4 = work.tile([64, QC], F16, tag="t2")
                nc.vector.tensor_mul(t3, kvts[0:64, :], ctab_s[0:64, ls])
                nc.vector.tensor_mul(t4, kts, s3tab_s[0:64, ls])
                nc.vector.tensor_add(kt2[0:64, ls], t3, t4)
                nc.sync.dma_start(out=kt2[64:128, ls], in_=kt2[0:64, ls])
                # V natural layout via PE transpose: kvts[64:128] is V.T [64, 512]
                for t in range(4):
                    vt_ps = mp.tile([128, 64], F16, tag="mp")
                    nc.tensor.transpose(vt_ps, kvts[64:128, 128 * t:128 * t + 128],
                                        identlo_s[64:128, :])
                    blk = 4 * lc + t
                    nc.vector.tensor_copy(vn[:, 65 * blk:65 * blk + 64], vt_ps)

            def make_chunk(qc):
                qs = slice(QC * qc, QC * qc + QC)
                nkb = 4 * (qc + 1)
                state = {}

                def qk(kb):
                    ks = slice(KB * kb, KB * kb + KB)
                    st = stp.tile([128, 2 * QC], F32, tag="st")
                    nc.tensor.matmul(st[:, 0:QC], kt2[0:64, ks], qtrope[0:64, qs],
                                     start=True, stop=True)
                    nc.tensor.matmul(st[:, QC:2 * QC], kt2[64:128, ks], qtrope[64:128, qs],
                                     start=True, stop=True)
                    pt = ptp.tile([128, 2 * QC], F16, tag="pt")
                    nc.scalar.activation(pt, st, mybir.ActivationFunctionType.Exp,
                                         scale=0.125)
                    m = kb - 4 * qc
                    if m >= 0:
                        lo = KB * m
                        nc.vector.tensor_mul(pt[:, lo:lo + KB], pt[:, lo:lo + KB], tri_s)
                        nc.vector.tensor_mul(pt[:, QC + lo:QC + lo + KB],
                                             pt[:, QC + lo:QC + lo + KB], tri_s)
                    return pt

                def pv(kb, pt):
                    if kb == 0:
                        state["ot0"] = otp.tile([65, QC], F32, tag="ot", name="ot0")
                        state["ot1"] = otp.tile([65, QC], F32, tag="ot", name="ot1")
                    m = kb - 4 * qc
                    lo = KB * m if m >= 0 else 0
                    vblk = vn[:, 65 * kb:65 * kb + 65]
                    nc.tensor.matmul(state["ot0"][:, lo:QC], vblk, pt[:, lo:QC],
                                     start=(kb == 0), stop=(kb == nkb - 1),
                                     skip_group_check=True)
                    nc.tensor.matmul(state["ot1"][:, lo:QC], vblk, pt[:, QC + lo:2 * QC],
                                     start=(kb == 0), stop=(kb == nkb - 1),
                                     skip_group_check=True)

                def finish_a(eng=None):
                    eng = eng or nc.gpsimd
                    rcs = []
                    for h, ot in enumerate((state["ot0"], state["ot1"])):
                        dst = work.tile([128, QC], F32, tag="dst")
                        nc.vector.tensor_copy(dst[64:65, :], ot[64:65, :])
                        dn = work.tile([128, 4], F32, tag="dn")
                        eng.dma_start(out=dn, in_=dst[64:65, :])
                        rc = work.tile([128, 4], F16, tag="rc")
                        with nc.allow_low_precision(reason="softmax denom recip fp16"):
                            nc.vector.reciprocal(rc, dn)
                        rrow = work.tile([1, QC], F16, tag="rrow")
                        eng.dma_start(out=rrow, in_=rc)
                        rcs.append(rrow)
                    state["rcs"] = rcs

                def finish_b():
                    otns = []
                    for h, ot in enumerate((state["ot0"], state["ot1"])):
                        rbc_ps = mp.tile([64, QC], F32, tag="mp")
                        nc.tensor.matmul(rbc_ps, ones64_s, state["rcs"][h],
                                         start=True, stop=True)
                        rbc = work.tile([64, QC], F32, tag="rbc")
                        nc.vector.tensor_copy(rbc, rbc_ps)
                        otn = work.tile([64, QC], F16, tag=f"otn{h}")
                        nc.vector.tensor_mul(otn, ot[0:64, :], rbc)
                        otns.append(otn)
                    for dc in range(8):
                        yps = mp.tile([128, QC], F32, tag="mp")
                        nc.tensor.matmul(yps, wo0_s[:, 128 * dc:128 * dc + 128], otns[0],
                                         start=True, stop=False)
                        nc.tensor.matmul(yps, wo1_s[:, 128 * dc:128 * dc + 128], otns[1],
                                         start=False, stop=True)
                        ysb = work.tile([128, QC], F16, tag="ysb")
                        nc.vector.tensor_copy(ysb, yps)
                        nc.sync.dma_start(out=yt[128 * dc:128 * dc + 128, qs], in_=ysb)

                return nkb, qk, pv, finish_a, finish_b

            nc.sync.dma_start(out=identlo_s, in_=identlo[:, :])
            proj_dma(0)
            proj_compute(0)
            load_late_consts()
            if LC > 1:
                proj_dma(1)
                proj_compute(1)
            if LC > 2:
                proj_dma(2)
            prev = None
            for qc in range(LC):
                nkb, qk, pv, finish_a, finish_b = make_chunk(qc)
                pts = {}
                pts[0] = qk(0)
                if nkb > 1:
                    pts[1] = qk(1)
                if prev is not None:
                    prev[0]()
                if qc + 3 < LC:
                    proj_dma(qc + 3)
                if qc + 2 < LC:
                    proj_compute(qc + 2)
                fb_done = prev is None
                fb_kb = min(8, nkb - 2)
                for kb in range(nkb):
                    if kb + 2 < nkb:
                        pts[kb + 2] = qk(kb + 2)
                    pv(kb, pts.pop(kb))
                    if kb == fb_kb and not fb_done:
                        prev[1]()
                        fb_done = True
                if not fb_done:
                    prev[1]()
                prev = (finish_a, finish_b)
            prev[0](nc.sync)
            prev[1]()

    nc.finalize()
    return nc


def prep_inputs(x, Wq, Wk, Wv, Wo, token_positions, L=4096):
    """Host-side sharding + layout prep. Returns per-core input maps."""
    x = np.asarray(x, dtype=np.float32)
    Wq = np.asarray(Wq, dtype=np.float32)
    Wk = np.asarray(Wk, dtype=np.float32)
    Wv = np.asarray(Wv, dtype=np.float32)
    Wo = np.asarray(Wo, dtype=np.float32)
    pos = np.asarray(token_positions)[0].astype(np.float64)

    xt = np.ascontiguousarray(x[0].T).astype(np.float16)   # [D, L]
    i = np.arange(HEAD_DIM // 2, dtype=np.float64)
    freq = THETA ** (-2.0 * i / HEAD_DIM)                  # [32]
    ang = pos[:, None] * freq[None, :]                     # [L, 32]
    cos = np.cos(ang).T
    sin = np.sin(ang).T
    c64 = np.concatenate([cos, cos], axis=0)               # [64, L]
    s64 = np.concatenate([-sin, sin], axis=0)
    ctab = np.ascontiguousarray(np.concatenate([c64, c64], axis=0)).astype(np.float16)
    s3tab = np.ascontiguousarray(np.concatenate([s64, s64], axis=0)).astype(np.float16)

    perm = np.concatenate([np.arange(0, 64, 2), np.arange(1, 64, 2)])
    tri = (np.arange(128)[None, :] >= np.arange(128)[:, None]).astype(np.float16)
    tri = np.ascontiguousarray(tri)
    ones64 = np.ones((1, 64), dtype=np.float16)
    identlo = np.zeros((128, 64), dtype=np.float16)
    identlo[np.arange(128), np.arange(128) % 64] = 1.0

    in_maps = []
    for c in range(N_CORES):
        h0, h1, g = 2 * c, 2 * c + 1, c // 2
        qrows = np.concatenate([64 * h0 + perm, 64 * h1 + perm])
        wqt = np.ascontiguousarray(Wq[qrows, :].T).astype(np.float16)
        kv = np.concatenate([Wk[64 * g + perm, :], Wv[64 * g:64 * g + 64, :]], axis=0)
        wkvt = np.ascontiguousarray(kv.T).astype(np.float16)
        wo0 = np.ascontiguousarray(Wo[:, 64 * h0:64 * h0 + 64].T).astype(np.float16)
        wo1 = np.ascontiguousarray(Wo[:, 64 * h1:64 * h1 + 64].T).astype(np.float16)
        in_maps.append(dict(xt=xt, wqt=wqt, wkvt=wkvt, wo0=wo0, wo1=wo1,
                            ctab=ctab, s3tab=s3tab, tri=tri,
                            identlo=identlo, ones64=ones64))
    return in_maps


_NC_CACHE = {}


def _get_nc(L=4096):
    if L not in _NC_CACHE:
        _NC_CACHE[L] = build_kernel(L)
    return _NC_CACHE[L]


def kernel(x, Wq, Wk, Wv, Wo, token_positions):
    B, L, D = np.asarray(x).shape
    nc = _get_nc(L)
    in_maps = prep_inputs(x, Wq, Wk, Wv, Wo, token_positions, L=L)
    res = run_bass_kernel_spmd(nc, in_maps, list(range(N_CORES)))
    y = np.zeros((D_MODEL, L), dtype=np.float32)
    for r in res.results:
        y += r["yt"].astype(np.float32)
    return np.ascontiguousarray(y.T)[None].astype(np.float32)


# revision 35
# speedup vs baseline: 1.0743x; 1.0251x over previous
"""Trainium2 Bass kernel: GQA multi-head self-attention (B=1, L=4096, D=1024,
16 Q heads, 4 KV heads, head_dim 64, interleaved RoPE, causal softmax).

Sharding: 2 query heads + their (shared) KV head per core, 8 cores.
Each core computes a full-shape partial output Y_c.T = (attn_c @ Wo_c.T).T
(Megatron row-parallel style); the host sums the 8 partials.

Device-side design (per core):
  - x is fed pre-transposed (xT [D, L], fp16) so projection matmuls stream
    natural SBUF tiles; matmul operands are fp16 or fp32r (1 cycle/row on the
    PE, vs 4 for plain fp32), accumulation stays fp32 in PSUM.
  - Q.T/K.T are produced in a "half-split" head-dim order (even dims then odd
    dims per head, via host-permuted weight rows) so RoPE's rotate-pair becomes
    a 32-partition block swap, done with SBUF->SBUF DMAs.
  - Attention runs in the S.T = K @ Q.T orientation: scores land in PSUM as
    [k=128, q=512] tiles (both heads side by side in one 2-bank tile, via
    row-group-packed K=64 matmuls), exp runs on the scalar engine straight out
    of PSUM, and PV uses [V | ones] as the stationary operand so softmax
    denominators come out as row 64 of the PV accumulator for free.
  - No max-subtraction pass: scores are O(1) here, exp cannot overflow, and
    softmax is shift-invariant so the result matches the reference.
  - Emission is software-pipelined: QK^T/exp run two key-blocks ahead of PV,
    and each chunk's normalize + output projection is deferred until the next
    chunk's first two key-blocks are in flight.
"""

import sys

for _p in ("/opt/trn_rl_repo",):
    if _p not in sys.path:
        sys.path.insert(0, _p)

import numpy as np

import concourse.bacc as bacc
import concourse.mybir as mybir
import concourse.tile as tile
from concourse.bass_utils import run_bass_kernel_spmd

F32 = mybir.dt.float32
F16 = mybir.dt.float16

D_MODEL = 1024
NUM_HEADS = 16
NUM_KV_HEADS = 4
HEAD_DIM = 64
THETA = 10000.0
N_CORES = 8
QC = 512          # query chunk (free dim of S.T tiles per head)
KB = 128          # key block (partition dim of S.T tiles)


def build_kernel(L=4096):
    """One-core SPMD program. Handles its 2 query heads + 1 shared KV head."""
    nc = bacc.Bacc(None, target_bir_lowering=False)
    LC = L // QC          # number of 512-wide l/q chunks
    NT = L // KB          # number of 128-row key blocks / V tiles

    xt = nc.dram_tensor("xt", [D_MODEL, L], F16, kind="ExternalInput")
    wqt = nc.dram_tensor("wqt", [D_MODEL, 128], F16, kind="ExternalInput")
    wkvt = nc.dram_tensor("wkvt", [D_MODEL, 128], F16, kind="ExternalInput")
    wo0 = nc.dram_tensor("wo0", [64, D_MODEL], F16, kind="ExternalInput")
    wo1 = nc.dram_tensor("wo1", [64, D_MODEL], F16, kind="ExternalInput")
    ctab = nc.dram_tensor("ctab", [128, L], F16, kind="ExternalInput")
    s3tab = nc.dram_tensor("s3tab", [128, L], F16, kind="ExternalInput")
    tri = nc.dram_tensor("tri", [128, 128], F16, kind="ExternalInput")
    identlo = nc.dram_tensor("identlo", [128, 64], F16, kind="ExternalInput")
    ones64 = nc.dram_tensor("ones64", [1, 64], F16, kind="ExternalInput")
    yt = nc.dram_tensor("yt", [D_MODEL, L], F16, kind="ExternalOutput")

    with tile.TileContext(nc) as tc:
        with (
            tc.tile_pool(name="consts", bufs=1) as consts,
            tc.tile_pool(name="big", bufs=1) as big,
            tc.tile_pool(name="xin", bufs=3) as xin,
            tc.tile_pool(name="work", bufs=4) as work,
            tc.tile_pool(name="ptp", bufs=12) as ptp,
            tc.tile_pool(name="stp", bufs=2, space="PSUM") as stp,
            tc.tile_pool(name="otp", bufs=2, space="PSUM") as otp,
            tc.tile_pool(name="mp", bufs=2, space="PSUM") as mp,
        ):
            # ---- constants in SBUF ----
            wqt_s = consts.tile([128, 8, 128], F16, tag="wqt")
            wkvt_s = consts.tile([128, 8, 128], F16, tag="wkvt")
            wo0_s = consts.tile([64, D_MODEL], F16, tag="wo0")
            wo1_s = consts.tile([64, D_MODEL], F16, tag="wo1")
            ctab_s = consts.tile([128, L], F16, tag="ctab")
            s3tab_s = consts.tile([128, L], F16, tag="s3tab")
            ones64_s = consts.tile([1, 64], F16, tag="ones64")
            tri_s = consts.tile([128, 128], F16, tag="tri")
            identlo_s = consts.tile([128, 64], F16, tag="identlo")

            def load_late_consts():
                nc.sync.dma_start(out=wo0_s, in_=wo0[:, :])
                nc.sync.dma_start(out=wo1_s, in_=wo1[:, :])
                nc.sync.dma_start(out=ones64_s, in_=ones64[:, :])
                nc.sync.dma_start(out=tri_s, in_=tri[:, :])

            # ---- persistent per-core activations ----
            qtrope = big.tile([128, L], F16, tag="qtrope")      # [2*64 halfsplit d, L]
            kt2 = big.tile([128, L], F16, tag="kt2")            # K.T duplicated twice
            vn = big.tile([128, NT * 65], F16, tag="vn")        # [V | 1] blocks
            nc.gpsimd.memset(vn, 1.0)

            xtiles = {}

            xt_r = xt.rearrange("(dc p) l -> p dc l", p=128)      # [128, 8, L]

            def proj_dma(lc):
                ls = slice(QC * lc, QC * lc + QC)
                if lc == 0:
                    nc.sync.dma_start(out=wqt_s,
                                      in_=wqt.rearrange("(dc p) m -> p dc m", p=128))
                    nc.sync.dma_start(out=wkvt_s,
                                      in_=wkvt.rearrange("(dc p) m -> p dc m", p=128))
                xbig = xin.tile([128, 8, QC], F16, tag="xt")
                nc.sync.dma_start(out=xbig, in_=xt_r[:, :, ls])
                nc.sync.dma_start(out=ctab_s[:, ls], in_=ctab[:, ls])
                nc.sync.dma_start(out=s3tab_s[:, ls], in_=s3tab[:, ls])
                xtiles[lc] = xbig

            def proj_compute(lc):
                ls = slice(QC * lc, QC * lc + QC)
                xbig = xtiles.pop(lc)
                qt_ps = mp.tile([128, QC], F32, tag="mp")
                kvt_ps = mp.tile([128, QC], F32, tag="mp")
                for dc in range(8):
                    nc.tensor.matmul(qt_ps, wqt_s[:, dc, :], xbig[:, dc, :],
                                     start=(dc == 0), stop=(dc == 7))
                for dc in range(8):
                    nc.tensor.matmul(kvt_ps, wkvt_s[:, dc, :], xbig[:, dc, :],
                                     start=(dc == 0), stop=(dc == 7))
                # evacuate PSUM (fp32 -> fp16)
                qtraw = work.tile([128, QC], F16, tag="qtraw")
                kvts = work.tile([128, QC], F16, tag="kvts")
                nc.vector.tensor_copy(qtraw, qt_ps)
                nc.vector.tensor_copy(kvts, kvt_ps)
                # half-split pair swap via SBUF->SBUF DMA (32-row blocks)
                qts = work.tile([128, QC], F16, tag="qts")
                for (a, b) in ((0, 32), (32, 0), (64, 96), (96, 64)):
                    nc.sync.dma_start(out=qts[a:a + 32, :], in_=qtraw[b:b + 32, :])
                kts = work.tile([64, QC], F16, tag="kts")
                nc.sync.dma_start(out=kts[0:32, :], in_=kvts[32:64, :])
                nc.sync.dma_start(out=kts[32:64, :], in_=kvts[0:32, :])
                # RoPE: rot = raw*C + swapped*S3
                t1 = work.tile([128, QC], F16, tag="t1")
                t2 = work.tile([128, QC], F16, tag="t2")
                nc.vector.tensor_mul(t1, qtraw, ctab_s[:, ls])
                nc.vector.tensor_mul(t2, qts, s3tab_s[:, ls])
                nc.vector.tensor_add(qtrope[:, ls], t1, t2)
                t3 = work.tile([64, QC], F16, tag="t1")
                t4 = work.tile([64, QC], F16, tag="t2")
                nc.vector.tensor_mul(t3, kvts[0:64, :], ctab_s[0:64, ls])
                nc.vector.tensor_mul(t4, kts, s3tab_s[0:64, ls])
                nc.vector.tensor_add(kt2[0:64, ls], t3, t4)
                nc.sync.dma_start(out=kt2[64:128, ls], in_=kt2[0:64, ls])
                # V natural layout via PE transpose: kvts[64:128] is V.T [64, 512]
                for t in range(4):
                    vt_ps = mp.tile([128, 64], F16, tag="mp")
                    nc.tensor.transpose(vt_ps, kvts[64:128, 128 * t:128 * t + 128],
                                        identlo_s[64:128, :])
                    blk = 4 * lc + t
                    nc.vector.tensor_copy(vn[:, 65 * blk:65 * blk + 64], vt_ps)

            def make_chunk(qc):
                qs = slice(QC * qc, QC * qc + QC)
                nkb = 4 * (qc + 1)
                state = {}

                def qk(kb):
                    ks = slice(KB * kb, KB * kb + KB)
                    st = stp.tile([128, 2 * QC], F32, tag="st")
                    nc.tensor.matmul(st[:, 0:QC], kt2[0:64, ks], qtrope[0:64, qs],
                                     start=True, stop=True)
                    nc.tensor.matmul(st[:, QC:2 * QC], kt2[64:128, ks], qtrope[64:128, qs],
                                     start=True, stop=True)
                    pt = ptp.tile([128, 2 * QC], F16, tag="pt")
                    nc.scalar.activation(pt, st, mybir.ActivationFunctionType.Exp,
                                         scale=0.125)
                    m = kb - 4 * qc
                    if m >= 0:
                        lo = KB * m
                        nc.vector.tensor_mul(pt[:, lo:lo + KB], pt[:, lo:lo + KB], tri_s)
                        nc.vector.tensor_mul(pt[:, QC + lo:QC + lo + KB],
                                             pt[:, QC + lo:QC + lo + KB], tri_s)
                    return pt

                def pv(kb, pt):
                    if kb == 0:
                        state["ot0"] = otp.tile([65, QC], F32, tag="ot", name="ot0")
                        state["ot1"] = otp.tile([65, QC], F32, tag="ot", name="ot1")
                    m = kb - 4 * qc
                    lo = KB * m if m >= 0 else 0
                    vblk = vn[:, 65 * kb:65 * kb + 65]
                    nc.tensor.matmul(state["ot0"][:, lo:QC], vblk, pt[:, lo:QC],
                                     start=(kb == 0), stop=(kb == nkb - 1),
                                     skip_group_check=True)
                    nc.tensor.matmul(state["ot1"][:, lo:QC], vblk, pt[:, QC + lo:2 * QC],
                                     start=(kb == 0), stop=(kb == nkb - 1),
                                     skip_group_check=True)

                def finish_a(eng=None):
                    eng = eng or nc.gpsimd
                    rcs = []
                    for h, ot in enumerate((state["ot0"], state["ot1"])):
                        dst = work.tile([128, QC], F32, tag="dst")
                        nc.vector.tensor_copy(dst[64:65, :], ot[64:65, :])
                        dn = work.tile([128, 4], F32, tag="dn")
                        eng.dma_start(out=dn, in_=dst[64:65, :])
                        rc = work.tile([128, 4], F16, tag="rc")
                        with nc.allow_low_precision(reason="softmax denom recip fp16"):
                            nc.vector.reciprocal(rc, dn)
                        rrow = work.tile([1, QC], F16, tag="rrow")
                        eng.dma_start(out=rrow, in_=rc)
                        rcs.append(rrow)
                    state["rcs"] = rcs

                def finish_b():
                    otns = []
                    for h, ot in enumerate((state["ot0"], state["ot1"])):
                        rbc_ps = mp.tile([64, QC], F32, tag="mp")
                        nc.tensor.matmul(rbc_ps, ones64_s, state["rcs"][h],
                                         start=True, stop=True)
                        rbc = work.tile([64, QC], F32, tag="rbc")
                        nc.vector.tensor_copy(rbc, rbc_ps)
                        otn = work.tile([64, QC], F16, tag=f"otn{h}")
                        nc.vector.tensor_mul(otn, ot[0:64, :], rbc)
                        otns.append(otn)
                    for dc in range(8):
                        yps = mp.tile([128, QC], F32, tag="mp")
                        nc.tensor.matmul(yps, wo0_s[:, 128 * dc:128 * dc + 128], otns[0],
                                         start=True, stop=False)
                        nc.tensor.matmul(yps, wo1_s[:, 128 * dc:128 * dc + 128], otns[1],
                                         start=False, stop=True)
                        ysb = work.tile([128, QC], F16, tag="ysb")
                        nc.vector.tensor_copy(ysb, yps)
                        nc.sync.dma_start(out=yt[128 * dc:128 * dc + 128, qs], in_=ysb)

                return nkb, qk, pv, finish_a, finish_b

            nc.sync.dma_start(out=identlo_s, in_=identlo[:, :])
            proj_dma(0)
            proj_compute(0)
            load_late_consts()
            if LC > 1:
                proj_dma(1)
                proj_compute(1)
            if LC > 2:
                proj_dma(2)
            prev = None
            for qc in range(LC):
                nkb, qk, pv, finish_a, finish_b = make_chunk(qc)
                pts = {}
                pts[0] = qk(0)
                if nkb > 1:
                    pts[1] = qk(1)
                if prev is not None:
                    prev[0]()
                if qc + 3 < LC:
                    proj_dma(qc + 3)
                if qc + 2 < LC:
                    proj_compute(qc + 2)
                fb_done = prev is None
                fb_kb = min(8, nkb - 2)
                for kb in range(nkb):
                    if kb + 2 < nkb:
                        pts[kb + 2] = qk(kb + 2)
                    pv(kb, pts.pop(kb))
                    if kb == fb_kb and not fb_done:
                        prev[1]()
                        fb_done = True
                if not fb_done:
                    prev[1]()
                prev = (finish_a, finish_b)
            prev[0](nc.sync)
            prev[1]()

    nc.finalize()
    return nc


def prep_inputs(x, Wq, Wk, Wv, Wo, token_positions, L=4096):
    """Host-side sharding + layout prep. Returns per-core input maps."""
    x = np.asarray(x, dtype=np.float32)
    Wq = np.asarray(Wq, dtype=np.float32)
    Wk = np.asarray(Wk, dtype=np.float32)
    Wv = np.asarray(Wv, dtype=np.float32)
    Wo = np.asarray(Wo, dtype=np.float32)
    pos = np.asarray(token_positions)[0].astype(np.float64)

    xt = np.ascontiguousarray(x[0].T).astype(np.float16)   # [D, L]
    i = np.arange(HEAD_DIM // 2, dtype=np.float64)
    freq = THETA ** (-2.0 * i / HEAD_DIM)                  # [32]
    ang = pos[:, None] * freq[None, :]                     # [L, 32]
    cos = np.cos(ang).T
    sin = np.sin(ang).T
    c64 = np.concatenate([cos, cos], axis=0)               # [64, L]
    s64 = np.concatenate([-sin, sin], axis=0)
    ctab = np.ascontiguousarray(np.concatenate([c64, c64], axis=0)).astype(np.float16)
    s3tab = np.ascontiguousarray(np.concatenate([s64, s64], axis=0)).astype(np.float16)

    perm = np.concatenate([np.arange(0, 64, 2), np.arange(1, 64, 2)])
    tri = (np.arange(128)[None, :] >= np.arange(128)[:, None]).astype(np.float16)
    tri = np.ascontiguousarray(tri)
    ones64 = np.ones((1, 64), dtype=np.float16)
    identlo = np.zeros((128, 64), dtype=np.float16)
    identlo[np.arange(128), np.arange(128) % 64] = 1.0

    in_maps = []
    for c in range(N_CORES):
        h0, h1, g = 2 * c, 2 * c + 1, c // 2
        qrows = np.concatenate([64 * h0 + perm, 64 * h1 + perm])
        wqt = np.ascontiguousarray(Wq[qrows, :].T).astype(np.float16)
        kv = np.concatenate([Wk[64 * g + perm, :], Wv[64 * g:64 * g + 64, :]], axis=0)
        wkvt = np.ascontiguousarray(kv.T).astype(np.float16)
        wo0 = np.ascontiguousarray(Wo[:, 64 * h0:64 * h0 + 64].T).astype(np.float16)
        wo1 = np.ascontiguousarray(Wo[:, 64 * h1:64 * h1 + 64].T).astype(np.float16)
        in_maps.append(dict(xt=xt, wqt=wqt, wkvt=wkvt, wo0=wo0, wo1=wo1,
                            ctab=ctab, s3tab=s3tab, tri=tri,
                            identlo=identlo, ones64=ones64))
    return in_maps


_NC_CACHE = {}


def _get_nc(L=4096):
    if L not in _NC_CACHE:
        _NC_CACHE[L] = build_kernel(L)
    return _NC_CACHE[L]


def kernel(x, Wq, Wk, Wv, Wo, token_positions):
    B, L, D = np.asarray(x).shape
    nc = _get_nc(L)
    in_maps = prep_inputs(x, Wq, Wk, Wv, Wo, token_positions, L=L)
    res = run_bass_kernel_spmd(nc, in_maps, list(range(N_CORES)))
    y = np.zeros((D_MODEL, L), dtype=np.float32)
    for r in res.results:
        y += r["yt"].astype(np.float32)
    return np.ascontiguousarray(y.T)[None].astype(np.float32)
